# revision 20
# baseline (speedup 1.0000x reference)
"""Trainium2 Bass kernel for one dense transformer block (B=2, S=2048, D=1024,
16 q-heads / 4 kv-heads GQA, squared-ReLU MLP), data-parallel over 8 NeuronCores.

Sharding: core c = (b, j), b = c // 4, j = c % 4, owns q-token tiles
{j, j+4, j+8, j+12} (128 tokens each) of batch b. K/V are computed for the full
sequence on every core (no collectives). The kv range for own q-tile t is
padded to 512*(t+1); causality enforced with per-core 0/1 masks on the
diagonal 512-wide kv chunk.

v2: bf16 matmul datapath (fp32 PSUM accumulation), host-side input rmsnorm
(xn = x*attn_norm_w/rms1 fed from DRAM), Q/K/V emission interleaved with
lagged norm + rope stages, rope rotate-half via a PE matmul with signs baked
into the R matrix, per-head normalizers replicated with gpsimd
partition_broadcast (q_gain folded into a per-partition stt scalar),
Abs_reciprocal_sqrt / reciprocal_approx_fast for all normalizers, exps
batched to 1024 elem/partition, AV matmuls lagged one tile behind QK+exp,
software-pipelined fc->proj MLP with the post-norm tail overlapped.

Numerical identities (exact up to negligible eps rescaling):
  - per-head q/k rmsnorm is scale-invariant per token -> Q/K project from the
    host-normalized xn without correction
  - the MLP input rmsnorm cancels through relu()^2 -> proj -> post-rmsnorm
  - no softmax max-subtraction (logits bounded by |q||k|/8 = 8)
  - softmax denominator = ones-columns appended to V in the AV matmul
"""

import os

import numpy as np
import ml_dtypes

import concourse.bass as bass
from concourse import bacc
import concourse.tile as tile
import concourse.mybir as mybir
from concourse.bass_utils import run_bass_kernel_spmd

f32 = mybir.dt.float32
f32r = mybir.dt.float32r
bf16 = mybir.dt.bfloat16
AF = mybir.ActivationFunctionType
ALU = mybir.AluOpType

B, S, D = 2, 2048, 1024
H, HKV, HD = 16, 4, 64
MLP_HID = 4 * D
KV = HKV * HD
NT = 16
OWN = 512
EPS_BLOCK = 1e-6
EPS_QK = float(np.finfo(np.float32).eps)
ROPE_BASE = 10000.0

PAIRS = [(0, 4), (1, 5), (2, 6), (3, 7), (8, 12), (9, 13), (10, 14), (11, 15)]

USE_ABSRSQRT = os.environ.get("KERNEL_ABSRSQRT", "1") == "1"
USE_GPSIMD = os.environ.get("KERNEL_GPSIMD", "1") == "1"
INV_DT = bf16 if USE_ABSRSQRT else f32

PHASE_ORDER = ["c", "ab", "d", "e", "f"]


def build():
    max_ph = os.environ.get("KERNEL_PHASES", "f")
    ph_on = lambda p: PHASE_ORDER.index(p) <= PHASE_ORDER.index(max_ph)
    bacc.Bacc.move_matmul_waits_to_ldweights = lambda self: None
    nc = bacc.Bacc(None)

    def dram_in(name, shape, dt=bf16):
        return nc.dram_tensor(name, list(shape), dt, kind="ExternalInput")

    xT = dram_in("xT", (128, 8, S))              # normalized x, feature-major
    xq = dram_in("xq", (128, 8, OWN))            # normalized own x
    xres = dram_in("xres", (128, 8, OWN), f32)   # raw own x + attn bias
    xres2 = dram_in("xres2", (128, 8, OWN), f32) # xres + mlp bias
    wq = dram_in("wq", (8, 128, 8, 128))
    wk = dram_in("wk", (128, 8, KV))
    wv = dram_in("wv", (128, 8, KV))
    wo = dram_in("wo", (8, 128, 8, 128))
    wfc = dram_in("wfc", (32, 128, 8, 128))
    wproj = dram_in("wproj", (32, 2, 128, 4, 128))
    cosF = dram_in("cosF", (128, S))
    sinF = dram_in("sinF", (128, S))
    cosO = dram_in("cosO", (128, OWN))
    sinO = dram_in("sinO", (128, OWN))
    rmat = dram_in("rmat", (128, 128))           # rope rotate-half (signs baked)
    maskM = dram_in("maskM", (128, 4, 4, 128))
    ones_c = dram_in("ones_c", (128, 2))
    gq = dram_in("gq", (128, 8), f32)            # per-p-tile head gains
    g_attn = dram_in("g_attn", (128, 8), f32)
    g_mlp = dram_in("g_mlp", (128, 8), f32)

    out_t = nc.dram_tensor("out", [128, 8, OWN], f32, kind="ExternalOutput")

    def rsqrt_into(pool, out_ap, in_ap, bias_ap, pfx):
        """out_ap = 1/sqrt(in_ap/HD + eps); in_ap [2, n] psum."""
        if USE_ABSRSQRT:
            nc.scalar.activation(out_ap, in_ap, AF.Abs_reciprocal_sqrt,
                                 scale=1.0 / HD, bias=bias_ap)
        else:
            rms = pool.tile([2, in_ap.shape[-1]], f32, name=f"{pfx}rms",
                            tag=f"{pfx}rms")
            nc.scalar.activation(rms[:], in_ap, AF.Sqrt, scale=1.0 / HD,
                                 bias=bias_ap)
            nc.vector.reciprocal_approx_fast(out=out_ap, in_=rms[:])

    with tile.TileContext(nc) as tc, \
         tc.tile_pool(name="cst", bufs=1) as cst, \
         tc.tile_pool(name="big", bufs=1) as big:
        # --- constants / tables -------------------------------------------
        onesc = cst.tile([128, 2], bf16, tag="onesc")
        nc.sync.dma_start(onesc[:], ones_c[:])
        gqt = cst.tile([128, 8], f32, tag="gqt")
        nc.sync.dma_start(gqt[:], gq[:])
        rmt = cst.tile([128, 128], bf16, tag="rmt")
        epsq = cst.tile([128, 1], f32, tag="epsq")
        nc.vector.memset(epsq[:], EPS_QK)
        eps6 = cst.tile([128, 1], f32, tag="eps6")
        nc.vector.memset(eps6[:], EPS_BLOCK)
        gat = cst.tile([128, 8], f32, tag="gat")
        gml = cst.tile([128, 8], f32, tag="gml")

        from contextlib import ExitStack
        rope_stack = ExitStack()
        ropep = rope_stack.enter_context(tc.tile_pool(name="ropep", bufs=1))
        coso = ropep.tile([128, OWN], bf16, tag="coso")
        sino = ropep.tile([128, OWN], bf16, tag="sino")
        cosf = ropep.tile([128, S], bf16, tag="cosf")
        sinf = ropep.tile([128, S], bf16, tag="sinf")

        kT = big.tile([128, 2, S], bf16, tag="kT")
        v_all = big.tile([128, 4, NT, 66], bf16, tag="v_all")
        qT = big.tile([128, 8, OWN], bf16, tag="qT_xpr")
        y_all = big.tile([128, 8, OWN], bf16, tag="yall")
        xrs = big.tile([128, 8, OWN], f32, tag="xrs")
        xrs2 = big.tile([128, 8, OWN], f32, tag="xrs2_mout")

        # ------------- Phases C+AB interleaved: Q, K, V -------------------
        if ph_on("c"):
            nc.vector.tensor_copy(
                v_all[:, :, :, 64:66],
                onesc[:, 0, None, None].to_broadcast([128, 4, NT, 2]))
            with tc.tile_pool(name="px", bufs=1) as px, \
                 tc.tile_pool(name="pxc", bufs=2) as pxc, \
                 tc.tile_pool(name="pw", bufs=1) as pw, \
                 tc.tile_pool(name="pwq", bufs=3) as pwq, \
                 tc.tile_pool(name="psb", bufs=3) as psb, \
                 tc.tile_pool(name="pqn", bufs=1) as pqn, \
                 tc.tile_pool(name="pps", bufs=1, space="PSUM") as pps, \
                 tc.tile_pool(name="pss", bufs=2, space="PSUM") as pss, \
                 tc.tile_pool(name="prt", bufs=2, space="PSUM") as prt:
                # critical-path DMAs first: first chunk + V/K weights + xq
                xcs = {}
                xc0 = pxc.tile([128, 8, 512], bf16, tag="xc")
                nc.sync.dma_start(xc0[:], xT[:, :, 0:512])
                xcs[0] = xc0
                wvs = pw.tile([128, 8, KV], bf16, tag="wvs")
                nc.sync.dma_start(wvs[:], wv[:])
                wks = pw.tile([128, 8, KV], bf16, tag="wks")
                nc.sync.dma_start(wks[:], wk[:])
                xqs = px.tile([128, 8, OWN], bf16, tag="xqs")
                nc.sync.dma_start(xqs[:], xq[:])
                # long-lead tables and residuals after the critical batch
                nc.sync.dma_start(rmt[:], rmat[:])
                nc.sync.dma_start(coso[:], cosO[:])
                nc.sync.dma_start(sino[:], sinO[:])
                nc.sync.dma_start(cosf[:], cosF[:])
                nc.sync.dma_start(sinf[:], sinF[:])
                nc.sync.dma_start(xrs[:], xres[:])
                nc.sync.dma_start(xrs2[:], xres2[:])
                nc.sync.dma_start(gat[:], g_attn[:])
                nc.sync.dma_start(gml[:], g_mlp[:])

                def unit_v(ci):
                    # V token-major for the 4 token tiles of chunk ci
                    xc = xcs[ci]
                    for kt in range(4):
                        gkt = ci * 4 + kt
                        vps = pps.tile([128, KV], f32, tag="vps", bufs=1)
                        for k in range(8):
                            nc.tensor.matmul(vps[:], xc[:, k, kt * 128:(kt + 1) * 128],
                                             wvs[:, k, :], start=(k == 0), stop=(k == 7))
                        nc.scalar.activation(
                            v_all[:, :, gkt, 0:64],
                            vps[:].rearrange("p (g d) -> p g d", g=4), AF.Copy)

                def unit_proj(u):
                    kind = u[0]
                    if kind == "Q":
                        p = u[1]
                        wqs = pwq.tile([128, 8, 128], bf16, tag="wqs")
                        nc.sync.dma_start(wqs[:], wq[p])
                        ps = pps.tile([128, OWN], f32, tag="pps", bufs=3)
                        for k in range(8):
                            nc.tensor.matmul(ps[:], wqs[:, k, :], xqs[:, k, :],
                                             start=(k == 0), stop=(k == 7))
                    else:
                        ci, kp = u[1], u[2]
                        xc = xcs[ci]
                        ps = pps.tile([128, OWN], f32, tag="pps", bufs=3)
                        for k in range(8):
                            nc.tensor.matmul(ps[:], wks[:, k, kp * 128:(kp + 1) * 128],
                                             xc[:, k, :], start=(k == 0), stop=(k == 7))
                    sq = psb.tile([128, OWN], bf16, tag="sq")
                    nc.scalar.activation(sq[:], ps[:], AF.Square)
                    return (u, ps, sq)

                def unit_norm(st):
                    u, ps, sq = st
                    inv = psb.tile([2, 2, OWN], INV_DT, tag="inv")
                    for half in range(2):
                        hs = slice(half * 64, (half + 1) * 64)
                        ss = pss.tile([2, OWN], f32, tag="ss")
                        nc.tensor.matmul(ss[:], onesc[hs, 0:2], sq[hs, :],
                                         start=True, stop=True,
                                         tile_position=(half * 64, 0))
                        rsqrt_into(psb, inv[0:2, half, :], ss[0:2, :],
                                   epsq[0:2, :], "n")
                    rep = psb.tile([128, 2, OWN], INV_DT, tag="rep")
                    nc.gpsimd.partition_broadcast(rep[:], inv[0:1, :, :],
                                                  channels=128)
                    if u[0] == "Q":
                        xn = pqn.tile([128, OWN], bf16, tag="qn", bufs=8)
                        for half in range(2):
                            hs = slice(half * 64, (half + 1) * 64)
                            nc.vector.scalar_tensor_tensor(
                                xn[hs, :], ps[hs, :], gqt[hs, u[1], None],
                                rep[hs, half, :], ALU.mult, ALU.mult)
                    else:
                        xn = pqn.tile([128, OWN], bf16, tag="kn", bufs=4)
                        for half in range(2):
                            hs = slice(half * 64, (half + 1) * 64)
                            nc.vector.tensor_tensor(xn[hs, :], ps[hs, :],
                                                    rep[hs, half, :], ALU.mult)
                    return (u, xn)

                def unit_rope(st):
                    u, xn = st
                    rot = prt.tile([128, OWN], f32, tag="rot")
                    nc.tensor.matmul(rot[:], rmt[:], xn[:], start=True, stop=True)
                    t1 = psb.tile([128, OWN], bf16, tag="t1")
                    t2 = psb.tile([128, OWN], bf16, tag="t2")
                    if u[0] == "Q":
                        nc.vector.tensor_tensor(t1[:], xn[:], coso[:], ALU.mult)
                        nc.vector.tensor_tensor(t2[:], rot[:], sino[:], ALU.mult)
                        nc.vector.tensor_tensor(qT[:, u[1], :], t1[:], t2[:],
                                                ALU.add)
                    else:
                        ci, kp = u[1], u[2]
                        sl = slice(ci * 512, (ci + 1) * 512)
                        nc.vector.tensor_tensor(t1[:], xn[:], cosf[:, sl], ALU.mult)
                        nc.vector.tensor_tensor(t2[:], rot[:], sinf[:, sl], ALU.mult)
                        nc.vector.tensor_tensor(kT[:, kp, sl], t1[:], t2[:],
                                                ALU.add)

                units = []
                qp = 0
                for ci in range(4):
                    units.append(("V", ci))
                    units.append(("K", ci, 0))
                    units.append(("K", ci, 1))
                    units.append(("Q", qp)); qp += 1
                    units.append(("Q", qp)); qp += 1

                normq, ropeq = [], []
                for u in units:
                    if u[0] == "V":
                        ci = u[1]
                        if ci + 1 < 4:
                            xcn = pxc.tile([128, 8, 512], bf16, tag="xc")
                            nc.sync.dma_start(xcn[:],
                                              xT[:, :, (ci + 1) * 512:(ci + 2) * 512])
                            xcs[ci + 1] = xcn
                        unit_v(ci)
                        continue
                    normq.append(unit_proj(u))
                    if len(normq) >= 2:
                        ropeq.append(unit_norm(normq.pop(0)))
                    if len(ropeq) >= 3:
                        unit_rope(ropeq.pop(0))
                while normq:
                    ropeq.append(unit_norm(normq.pop(0)))
                while ropeq:
                    unit_rope(ropeq.pop(0))

        # ------------- Phase D: attention ---------------------------------
        if ph_on("d"):
            with tc.tile_pool(name="pd_m", bufs=1) as pd_m, \
                 tc.tile_pool(name="pd_pt", bufs=6) as pd_pt, \
                 tc.tile_pool(name="pd_sb", bufs=3) as pd_sb, \
                 tc.tile_pool(name="pd_s", bufs=2, space="PSUM") as pd_s, \
                 tc.tile_pool(name="pd_y", bufs=2, space="PSUM") as pd_y:
                masks = pd_m.tile([128, 4, 4, 128], bf16, tag="masks")
                nc.sync.dma_start(masks[:], maskM[:])
                for t in range(4):
                    qsl = slice(t * 128, (t + 1) * 128)
                    n_chunks = t + 1
                    n_kvt = 4 * n_chunks
                    for half in range(2):
                        gA, gB = 2 * half, 2 * half + 1
                        yA = pd_y.tile([66, 4, 128], f32, tag="yA")
                        yB = pd_y.tile([66, 4, 128], f32, tag="yB")
                        qsA = qT[0:64, 4 * half:4 * half + 4, qsl]
                        qsB = qT[64:128, 4 * half:4 * half + 4, qsl]
                        av_pend = []

                        def emit_av(c, i2, ptA, ptB):
                            for isub in range(2):
                                kvt = 4 * c + 2 * i2 + isub
                                nc.tensor.matmul(yA[:], v_all[:, gA, kvt, :],
                                                 ptA[:, isub, :, :],
                                                 start=(kvt == 0),
                                                 stop=(kvt == n_kvt - 1))
                                nc.tensor.matmul(yB[:], v_all[:, gB, kvt, :],
                                                 ptB[:, isub, :, :],
                                                 start=(kvt == 0),
                                                 stop=(kvt == n_kvt - 1))

                        for c in range(n_chunks):
                            for i2 in range(2):
                                sA = pd_s.tile([128, 2, 4, 128], f32, tag="s")
                                sB = pd_s.tile([128, 2, 4, 128], f32, tag="s")
                                for isub in range(2):
                                    i = 2 * i2 + isub
                                    ks = slice((4 * c + i) * 128,
                                               (4 * c + i + 1) * 128)
                                    nc.tensor.matmul(sA[:, isub, :, :],
                                                     kT[0:64, half, ks], qsA,
                                                     start=True, stop=True,
                                                     tile_position=(0, 0))
                                    nc.tensor.matmul(sB[:, isub, :, :],
                                                     kT[64:128, half, ks], qsB,
                                                     start=True, stop=True,
                                                     tile_position=(64, 0))
                                ptA = pd_pt.tile([128, 2, 4, 128], bf16, tag="ptA")
                                ptB = pd_pt.tile([128, 2, 4, 128], bf16, tag="ptB")
                                nc.scalar.activation(ptA[:], sA[:], AF.Exp,
                                                     scale=0.125)
                                nc.scalar.activation(ptB[:], sB[:], AF.Exp,
                                                     scale=0.125)
                                if c == t:
                                    mbc = masks[:, t, 2 * i2:2 * i2 + 2, None, :] \
                                        .to_broadcast([128, 2, 4, 128])
                                    nc.vector.tensor_tensor(ptA[:], ptA[:], mbc,
                                                            ALU.mult)
                                    nc.vector.tensor_tensor(ptB[:], ptB[:], mbc,
                                                            ALU.mult)
                                av_pend.append((c, i2, ptA, ptB))
                                if len(av_pend) > 1:
                                    emit_av(*av_pend.pop(0))
                        while av_pend:
                            emit_av(*av_pend.pop(0))
                        for ab, y in ((0, yA), (1, yB)):
                            dn = pd_sb.tile([2, 4, 128], f32, tag="dn")
                            nc.vector.tensor_copy(dn[0:2, :, :], y[64:66, :, :])
                            invy = pd_sb.tile([2, 4, 128], f32, tag="invy")
                            nc.vector.reciprocal_approx_fast(
                                out=invy[:], in_=dn[:])
                            repy = pd_sb.tile([128, 4, 128], f32, tag="repy")
                            nc.gpsimd.partition_broadcast(
                                repy[:], invy[0:1, :, :], channels=128)
                            c20 = 4 * half + 2 * ab
                            y4 = y.rearrange("p (a b) q -> p a b q", b=2)
                            r4 = repy.rearrange("p (a b) q -> p a b q", b=2)
                            for ph2 in range(2):
                                nc.vector.tensor_tensor(
                                    y_all[ph2 * 64:ph2 * 64 + 64,
                                          c20:c20 + 2, qsl],
                                    y4[0:64, :, ph2, :],
                                    r4[0:64, :, ph2, :], ALU.mult)

        # ------------- Phase E: Wo + post-norm + residual -----------------
        if ph_on("e"):
            xpr = big.tile([128, 8, OWN], bf16, tag="qT_xpr")
            xpb = big.tile([128, 8, OWN], f32, tag="xpb")
            with tc.tile_pool(name="pe_sb", bufs=2) as pe_sb, \
                 tc.tile_pool(name="pe_ao", bufs=1) as pe_ao, \
                 tc.tile_pool(name="pe_w", bufs=3) as pe_w, \
                 tc.tile_pool(name="pe_ps", bufs=3, space="PSUM") as pe_ps, \
                 tc.tile_pool(name="pe_ps1", bufs=1, space="PSUM") as pe_ps1:
                ao = pe_ao.tile([128, 8, OWN], f32, tag="ao")
                ssa = pe_ps1.tile([2, OWN], f32, tag="ssa")
                a2l = []
                for o in range(8):
                    wos = pe_w.tile([128, 8, 128], bf16, tag="wos")
                    nc.sync.dma_start(wos[:], wo[o])
                    aps = pe_ps.tile([128, OWN], f32, tag="aps")
                    for k in range(8):
                        nc.tensor.matmul(aps[:], wos[:, k, :], y_all[:, k, :],
                                         start=(k == 0), stop=(k == 7))
                    nc.scalar.activation(ao[:, o, :], aps[:], AF.Copy)
                    a2 = pe_sb.tile([128, OWN], bf16, tag="a2", bufs=3)
                    nc.scalar.activation(a2[:], aps[:], AF.Square)
                    a2l.append(a2)
                    if o >= 1:
                        nc.tensor.matmul(ssa[:], onesc[:, 0:2], a2l[o - 1][:],
                                         start=(o == 1), stop=False)
                nc.tensor.matmul(ssa[:], onesc[:, 0:2], a2l[7][:],
                                 start=False, stop=True)
                inva = pe_sb.tile([2, OWN], INV_DT, tag="inva")
                if USE_ABSRSQRT:
                    nc.scalar.activation(inva[:], ssa[0:2, :],
                                         AF.Abs_reciprocal_sqrt,
                                         scale=1.0 / D, bias=eps6[0:2, :])
                else:
                    rmsa = pe_sb.tile([2, OWN], f32, tag="rmsa")
                    nc.scalar.activation(rmsa[:], ssa[0:2, :], AF.Sqrt,
                                         scale=1.0 / D, bias=eps6[0:2, :])
                    nc.vector.reciprocal_approx_fast(out=inva[:], in_=rmsa[:])
                repa = pe_sb.tile([128, OWN], INV_DT, tag="repa")
                nc.gpsimd.partition_broadcast(repa[:], inva[0:1, :], channels=128)
                tmps = []
                for o in range(8):
                    tmp = pe_sb.tile([128, OWN], f32, tag="tmpe", bufs=8)
                    nc.vector.scalar_tensor_tensor(
                        tmp[:], ao[:, o, :], gat[:, o, None], repa[:],
                        ALU.mult, ALU.mult)
                    nc.vector.tensor_tensor(xpr[:, o, :], tmp[:], xrs[:, o, :],
                                            ALU.add)
                    tmps.append(tmp)
                for o in range(8):
                    nc.vector.tensor_tensor(xpb[:, o, :], tmps[o][:],
                                            xrs2[:, o, :], ALU.add)

        # ------------- Phase F: MLP ---------------------------------------
        if ph_on("f"):
            mout = big.tile([128, 8, OWN], f32, tag="xrs2_mout")
            with tc.tile_pool(name="pf_h2", bufs=1) as pf_h2, \
                 tc.tile_pool(name="pf_sb", bufs=3) as pf_sb, \
                 tc.tile_pool(name="pf_wf", bufs=3) as pf_wf, \
                 tc.tile_pool(name="pf_wp", bufs=3) as pf_wp, \
                 tc.tile_pool(name="pf_ps", bufs=2, space="PSUM") as pf_ps, \
                 tc.tile_pool(name="pf_mo", bufs=1, space="PSUM") as pf_mo, \
                 tc.tile_pool(name="pf_ss", bufs=1, space="PSUM") as pf_ss:
                h2 = pf_h2.tile([128, 32, OWN], bf16, tag="h2")
                ssm = pf_ss.tile([2, OWN], f32, tag="ssm")

                def emit_fc(hc):
                    wfs = pf_wf.tile([128, 8, 128], bf16, tag="wfs")
                    nc.sync.dma_start(wfs[:], wfc[hc])
                    hps = pf_ps.tile([128, OWN], f32, tag="hps")
                    for k in range(8):
                        nc.tensor.matmul(hps[:], wfs[:, k, :], xpr[:, k, :],
                                         start=(k == 0), stop=(k == 7))
                    hr = pf_sb.tile([128, OWN], bf16, tag="hr")
                    nc.scalar.activation(hr[:], hps[:], AF.Relu)
                    nc.vector.tensor_tensor(h2[:, hc, :], hr[:], hr[:], ALU.mult)

                mo_all = []
                for ohalf in range(2):
                    mo_ps = [pf_mo.tile([128, OWN], f32, name=f"mo{oi}",
                                        tag=f"mo{oi}") for oi in range(4)]
                    mo_all.append(mo_ps)

                for hc in range(34):
                    if hc < 32:
                        emit_fc(hc)
                    if hc >= 2:
                        hp = hc - 2
                        wps = pf_wp.tile([128, 4, 128], bf16, tag="wps")
                        nc.sync.dma_start(wps[:], wproj[hp, 0])
                        for oi in range(4):
                            nc.tensor.matmul(mo_all[0][oi][:], wps[:, oi, :],
                                             h2[:, hp, :],
                                             start=(hp == 0), stop=(hp == 31))
                # ohalf0 copies/squares run on scalar during the proj1 matmuls
                m2l = []
                for oi in range(4):
                    nc.scalar.activation(mout[:, oi, :], mo_all[0][oi][:],
                                         AF.Copy)
                    m2 = pf_sb.tile([128, OWN], bf16, tag="m2", bufs=8)
                    nc.scalar.activation(m2[:], mo_all[0][oi][:], AF.Square)
                    m2l.append(m2)
                for hc in range(32):
                    wps = pf_wp.tile([128, 4, 128], bf16, tag="wps")
                    nc.sync.dma_start(wps[:], wproj[hc, 1])
                    for oi in range(4):
                        nc.tensor.matmul(mo_all[1][oi][:], wps[:, oi, :],
                                         h2[:, hc, :],
                                         start=(hc == 0), stop=(hc == 31))
                    if hc < 4:
                        nc.tensor.matmul(ssm[:], onesc[:, 0:2], m2l[hc][:],
                                         start=(hc == 0), stop=False)
                for oi in range(4):
                    m2 = pf_sb.tile([128, OWN], bf16, tag="m2", bufs=8)
                    nc.scalar.activation(m2[:], mo_all[1][oi][:], AF.Square)
                    m2l.append(m2)
                for o in range(4, 8):
                    nc.tensor.matmul(ssm[:], onesc[:, 0:2], m2l[o][:],
                                     start=False, stop=(o == 7))
                invm = pf_sb.tile([2, OWN], INV_DT, tag="invm")
                if USE_ABSRSQRT:
                    nc.scalar.activation(invm[:], ssm[0:2, :],
                                         AF.Abs_reciprocal_sqrt,
                                         scale=1.0 / D, bias=eps6[0:2, :])
                else:
                    rmsm = pf_sb.tile([2, OWN], f32, tag="rmsm")
                    nc.scalar.activation(rmsm[:], ssm[0:2, :], AF.Sqrt,
                                         scale=1.0 / D, bias=eps6[0:2, :])
                    nc.vector.reciprocal_approx_fast(out=invm[:], in_=rmsm[:])
                repm = pf_sb.tile([128, OWN], INV_DT, tag="repm")
                nc.gpsimd.partition_broadcast(repm[:], invm[0:1, :], channels=128)
                for o in range(8):
                    msrc = mout[:, o, :] if o < 4 else mo_all[1][o - 4][:]
                    tmp = pf_sb.tile([128, OWN], f32, tag="tmpf")
                    nc.vector.scalar_tensor_tensor(
                        tmp[:], msrc, gml[:, o, None], repm[:],
                        ALU.mult, ALU.mult)
                    outv = pf_sb.tile([128, OWN], f32, tag="outv")
                    nc.vector.tensor_tensor(outv[:], tmp[:], xpb[:, o, :],
                                            ALU.add)
                    nc.sync.dma_start(out_t[:, o, :], outv[:])

        rope_stack.close()

    nc.finalize()
    return nc


def _feat_major(a):
    """[F, T] -> device layout [128, F//128, T]."""
    F, T = a.shape
    return np.ascontiguousarray(a.reshape(F // 128, 128, T).transpose(1, 0, 2))


def _vec_dev(v):
    return np.ascontiguousarray(v.reshape(-1, 128).T)


def _bf(a):
    return np.ascontiguousarray(np.asarray(a, np.float32)).astype(ml_dtypes.bfloat16)


_CACHE = {}
_RUN_KW = {}



def kernel(x, attn_norm_w, mlp_norm_w, attn_post_norm_w, mlp_post_norm_w,
           attn_scale, mlp_scale, attn_mod_gain, attn_mod_bias,
           mlp_mod_gain, mlp_mod_bias, Wq, Wk, Wv, Wo, q_gain, fc_w, proj_w):
    x = np.asarray(x, np.float32)
    q_gain = np.asarray(q_gain, np.float32)

    if "nc" not in _CACHE:
        _CACHE["nc"] = build()
    nc = _CACHE["nc"]

    anw = np.asarray(attn_norm_w, np.float32)
    mnw = np.asarray(mlp_norm_w, np.float32)
    fc_eff = np.asarray(fc_w, np.float32) * mnw[None, :]

    # host-side input rmsnorm: xn = x * anw / rms1(x)
    ms1 = np.mean(np.square(x), axis=-1, keepdims=True)
    xn = (x * (1.0 / np.sqrt(ms1 + EPS_BLOCK))) * anw[None, None, :]

    # Wq columns permuted so p-tile p holds heads PAIRS[p] stacked (64+64)
    perm = np.zeros(D, np.int64)
    for p, (a, b) in enumerate(PAIRS):
        perm[p * 128:p * 128 + 64] = np.arange(a * 64, a * 64 + 64)
        perm[p * 128 + 64:(p + 1) * 128] = np.arange(b * 64, b * 64 + 64)
    WqTp = np.asarray(Wq, np.float32).T[:, perm]
    wq_dev = np.stack([_feat_major(WqTp[:, p * 128:(p + 1) * 128]) for p in range(8)])
    wk_dev = _feat_major(np.asarray(Wk, np.float32).T)
    wv_dev = _feat_major(np.asarray(Wv, np.float32).T)

    # Wo rows permuted to match y_all layout: chunk c2 = 4*half + 2*ab + i//2,
    # partition ph2*64+f  ->  original feature 64*(4*(2*half+ab) + i) + f,
    # with i = 2*(c2 % 2) + ph2.
    perm2 = np.zeros(D, np.int64)
    for c2 in range(8):
        halfg = c2 // 2          # kv-head index (2*half + ab)
        for ph2 in range(2):
            i = 2 * (c2 % 2) + ph2
            h_orig = 4 * halfg + i
            rows = np.arange(64)
            perm2[c2 * 128 + ph2 * 64 + rows] = 64 * h_orig + rows
    WoT = np.asarray(Wo, np.float32).T[perm2, :]
    wo_dev = np.stack([_feat_major(WoT[:, o * 128:(o + 1) * 128]) for o in range(8)])

    fcT = fc_eff.T
    wfc_dev = np.stack([_feat_major(fcT[:, h * 128:(h + 1) * 128]) for h in range(32)])
    projT = np.asarray(proj_w, np.float32).T                  # [4096, 1024]
    wproj_dev = np.ascontiguousarray(
        projT.reshape(32, 128, 2, 4, 128).transpose(0, 2, 1, 3, 4))

    # rope tables; sin sign-folded: x1-groups (even 32-blocks) get -sin
    inv_freq = 1.0 / (ROPE_BASE ** (np.arange(0, HD, 2, dtype=np.float32) / HD))
    tpos = np.arange(S, dtype=np.float32)
    freqs = np.outer(tpos, inv_freq).astype(np.float32)
    cosT = np.ascontiguousarray(np.tile(np.cos(freqs).T, (4, 1)))   # [128, S]
    sinN = np.ascontiguousarray(np.tile(np.sin(freqs).T, (4, 1)))
    # rope rotate-half matrix with signs: rot = R @ x (per 64-feature head)
    R = np.zeros((128, 128), np.float32)
    for p in range(128):
        if p % 64 < 32:
            R[p, p + 32] = 1.0
        else:
            R[p, p - 32] = -1.0
    rmat_h = np.ascontiguousarray(R.T)

    gat_v = (np.asarray(attn_post_norm_w, np.float32)
             * np.asarray(attn_mod_gain, np.float32)
             * np.asarray(attn_scale, np.float32))
    bat_v = np.asarray(attn_mod_bias, np.float32) * np.asarray(attn_scale, np.float32)
    gml_v = (np.asarray(mlp_post_norm_w, np.float32)
             * np.asarray(mlp_mod_gain, np.float32)
             * np.asarray(mlp_scale, np.float32))
    bml_v = np.asarray(mlp_mod_bias, np.float32) * np.asarray(mlp_scale, np.float32)

    gq_h = np.zeros((128, 8), np.float32)
    for p, (a, b) in enumerate(PAIRS):
        gq_h[0:64, p] = q_gain[a]
        gq_h[64:128, p] = q_gain[b]

    shared = {
        "wq": _bf(wq_dev), "wk": _bf(wk_dev), "wv": _bf(wv_dev),
        "wo": _bf(wo_dev), "wfc": _bf(wfc_dev), "wproj": _bf(wproj_dev),
        "cosF": _bf(cosT), "sinF": _bf(sinN), "rmat": _bf(rmat_h),
        "ones_c": _bf(np.ones((128, 2), np.float32)),
        "gq": gq_h,
        "g_attn": _vec_dev(gat_v), "g_mlp": _vec_dev(gml_v),
    }

    in_maps = []
    owners = []
    for c in range(8):
        b, j = c // 4, c % 4
        rows = np.concatenate(
            [np.arange((j + 4 * t) * 128, (j + 4 * t + 1) * 128) for t in range(4)])
        owners.append((b, rows))
        xnb = xn[b].T
        x_own_raw = x[b].T[:, rows]
        mask = np.zeros((4, 4, 128, 128), np.float32)
        for t in range(4):
            m = j + 4 * t
            q_idx = m * 128 + np.arange(128)
            for ktl in range(4):
                kv_idx = 512 * t + 128 * ktl + np.arange(128)
                mask[t, ktl] = (kv_idx[:, None] <= q_idx[None, :])
        m_in = {
            "xT": _bf(_feat_major(xnb)),
            "xq": _bf(_feat_major(xnb[:, rows])),
            "xres": _feat_major(x_own_raw + bat_v[:, None]),
            "xres2": _feat_major(x_own_raw + (bat_v + bml_v)[:, None]),
            "cosO": _bf(cosT[:, rows]),
            "sinO": _bf(sinN[:, rows]),
            "maskM": _bf(np.ascontiguousarray(mask.transpose(2, 0, 1, 3))),
        }
        m_in.update(shared)
        in_maps.append(m_in)

    res = run_bass_kernel_spmd(nc, in_maps, core_ids=list(range(8)),
                               **_RUN_KW)
    _CACHE["last_result"] = res

    out = np.empty((B, S, D), np.float32)
    for c in range(8):
        b, rows = owners[c]
        o = res.results[c]["out"]
        out[b, rows, :] = o.transpose(2, 1, 0).reshape(OWN, D)
    return out


# revision 21
# speedup vs baseline: 1.5655x; 1.5655x over previous
"""Trainium2 Bass kernel for one dense transformer block (B=2, S=2048, D=1024,
16 q-heads / 4 kv-heads GQA, squared-ReLU MLP), data-parallel over 8 NeuronCores.

Sharding: core c = (b, j), b = c // 4, j = c % 4, owns q-token tiles
{j, j+4, j+8, j+12} (128 tokens each) of batch b. K/V are computed for the full
sequence on every core (no collectives). The kv range for own q-tile t is
padded to 512*(t+1); causality enforced with per-core 0/1 masks on the
diagonal 512-wide kv chunk.

v2: bf16 matmul datapath (fp32 PSUM accumulation), host-side input rmsnorm
(xn = x*attn_norm_w/rms1 fed from DRAM), Q/K/V emission interleaved with
lagged norm + rope stages, rope rotate-half via a PE matmul with signs baked
into the R matrix, per-head normalizers replicated with gpsimd
partition_broadcast (q_gain folded into a per-partition stt scalar),
Abs_reciprocal_sqrt / reciprocal_approx_fast for all normalizers, exps
batched to 1024 elem/partition, AV matmuls lagged one tile behind QK+exp,
software-pipelined fc->proj MLP with the post-norm tail overlapped.

Numerical identities (exact up to negligible eps rescaling):
  - per-head q/k rmsnorm is scale-invariant per token -> Q/K project from the
    host-normalized xn without correction
  - the MLP input rmsnorm cancels through relu()^2 -> proj -> post-rmsnorm
  - no softmax max-subtraction (logits bounded by |q||k|/8 = 8)
  - softmax denominator = ones-columns appended to V in the AV matmul
"""

import os

import numpy as np
import ml_dtypes

import concourse.bass as bass
from concourse import bacc
import concourse.tile as tile
import concourse.mybir as mybir
from concourse.bass_utils import run_bass_kernel_spmd

f32 = mybir.dt.float32
f32r = mybir.dt.float32r
bf16 = mybir.dt.bfloat16
AF = mybir.ActivationFunctionType
ALU = mybir.AluOpType

B, S, D = 2, 2048, 1024
H, HKV, HD = 16, 4, 64
MLP_HID = 4 * D
KV = HKV * HD
NT = 16
OWN = 512
EPS_BLOCK = 1e-6
EPS_QK = float(np.finfo(np.float32).eps)
ROPE_BASE = 10000.0

PAIRS = [(0, 4), (1, 5), (2, 6), (3, 7), (8, 12), (9, 13), (10, 14), (11, 15)]

USE_ABSRSQRT = os.environ.get("KERNEL_ABSRSQRT", "1") == "1"
INV_DT = bf16 if USE_ABSRSQRT else f32

PHASE_ORDER = ["c", "ab", "d", "e", "f"]


def build():
    max_ph = os.environ.get("KERNEL_PHASES", "f")
    ph_on = lambda p: PHASE_ORDER.index(p) <= PHASE_ORDER.index(max_ph)
    bacc.Bacc.move_matmul_waits_to_ldweights = lambda self: None
    nc = bacc.Bacc(None)

    def dram_in(name, shape, dt=bf16):
        return nc.dram_tensor(name, list(shape), dt, kind="ExternalInput")

    xT = dram_in("xT", (128, 8, S))              # normalized x, feature-major
    xq = dram_in("xq", (128, 8, OWN))            # normalized own x
    xres = dram_in("xres", (128, 8, OWN), f32)   # raw own x + attn bias
    xres2 = dram_in("xres2", (128, 8, OWN), f32) # xres + mlp bias
    wq = dram_in("wq", (8, 128, 8, 128))
    wk = dram_in("wk", (128, 8, KV))
    wv = dram_in("wv", (128, 8, KV))
    wo = dram_in("wo", (8, 128, 8, 128))
    wfc = dram_in("wfc", (32, 128, 8, 128))
    wproj = dram_in("wproj", (32, 2, 128, 4, 128))
    cosF = dram_in("cosF", (128, S))
    sinF = dram_in("sinF", (128, S))
    cosO = dram_in("cosO", (128, OWN))
    sinO = dram_in("sinO", (128, OWN))
    rmat = dram_in("rmat", (128, 128))           # rope rotate-half (signs baked)
    maskM = dram_in("maskM", (128, 4, 4, 128))
    ones_c = dram_in("ones_c", (128, 2))
    gq = dram_in("gq", (128, 8), f32)            # per-p-tile head gains
    g_attn = dram_in("g_attn", (128, 8), f32)
    g_mlp = dram_in("g_mlp", (128, 8), f32)

    out_t = nc.dram_tensor("out", [128, 8, OWN], f32, kind="ExternalOutput")

    def rsqrt_into(pool, out_ap, in_ap, bias_ap, pfx):
        """out_ap = 1/sqrt(in_ap/HD + eps); in_ap [2, n] psum."""
        if USE_ABSRSQRT:
            nc.scalar.activation(out_ap, in_ap, AF.Abs_reciprocal_sqrt,
                                 scale=1.0 / HD, bias=bias_ap)
        else:
            rms = pool.tile([2, in_ap.shape[-1]], f32, name=f"{pfx}rms",
                            tag=f"{pfx}rms")
            nc.scalar.activation(rms[:], in_ap, AF.Sqrt, scale=1.0 / HD,
                                 bias=bias_ap)
            nc.vector.reciprocal_approx_fast(out=out_ap, in_=rms[:])

    with tile.TileContext(nc) as tc, \
         tc.tile_pool(name="cst", bufs=1) as cst, \
         tc.tile_pool(name="big", bufs=1) as big:
        # --- constants / tables -------------------------------------------
        onesc = cst.tile([128, 2], bf16, tag="onesc")
        nc.sync.dma_start(onesc[:], ones_c[:])
        gqt = cst.tile([128, 8], f32, tag="gqt")
        nc.sync.dma_start(gqt[:], gq[:])
        rmt = cst.tile([128, 128], bf16, tag="rmt")
        epsq = cst.tile([128, 1], f32, tag="epsq")
        nc.vector.memset(epsq[:], EPS_QK)
        eps6 = cst.tile([128, 1], f32, tag="eps6")
        nc.vector.memset(eps6[:], EPS_BLOCK)
        gat = cst.tile([128, 8], f32, tag="gat")
        gml = cst.tile([128, 8], f32, tag="gml")

        from contextlib import ExitStack
        rope_stack = ExitStack()
        ropep = rope_stack.enter_context(tc.tile_pool(name="ropep", bufs=1))
        coso = ropep.tile([128, OWN], bf16, tag="coso")
        sino = ropep.tile([128, OWN], bf16, tag="sino")
        cosf = ropep.tile([128, S], bf16, tag="cosf")
        sinf = ropep.tile([128, S], bf16, tag="sinf")

        kT = big.tile([128, 2, S], bf16, tag="kT")
        v_all = big.tile([128, 4, NT, 66], bf16, tag="v_all")
        qT = big.tile([128, 8, OWN], bf16, tag="qT_xpr")
        y_all = big.tile([128, 8, OWN], bf16, tag="yall")
        xrs = big.tile([128, 8, OWN], f32, tag="xrs")
        xrs2 = big.tile([128, 8, OWN], f32, tag="xrs2_mout")

        # ------------- Phases C+AB interleaved: Q, K, V -------------------
        if ph_on("c"):
            nc.vector.tensor_copy(
                v_all[:, :, :, 64:66],
                onesc[:, 0, None, None].to_broadcast([128, 4, NT, 2]))
            with tc.tile_pool(name="px", bufs=1) as px, \
                 tc.tile_pool(name="pxc", bufs=2) as pxc, \
                 tc.tile_pool(name="pw", bufs=1) as pw, \
                 tc.tile_pool(name="pwq", bufs=3) as pwq, \
                 tc.tile_pool(name="psb", bufs=3) as psb, \
                 tc.tile_pool(name="pqn", bufs=1) as pqn, \
                 tc.tile_pool(name="pps", bufs=1, space="PSUM") as pps, \
                 tc.tile_pool(name="pss", bufs=2, space="PSUM") as pss, \
                 tc.tile_pool(name="prt", bufs=2, space="PSUM") as prt:
                # critical-path DMAs first: first chunk + V/K weights + xq
                xcs = {}
                xc0 = pxc.tile([128, 8, 512], bf16, tag="xc")
                nc.sync.dma_start(xc0[:], xT[:, :, 0:512])
                xcs[0] = xc0
                wvs = pw.tile([128, 8, KV], bf16, tag="wvs")
                nc.sync.dma_start(wvs[:], wv[:])
                wks = pw.tile([128, 8, KV], bf16, tag="wks")
                nc.sync.dma_start(wks[:], wk[:])
                xqs = px.tile([128, 8, OWN], bf16, tag="xqs")
                nc.sync.dma_start(xqs[:], xq[:])
                # long-lead tables and residuals after the critical batch
                nc.sync.dma_start(rmt[:], rmat[:])
                nc.sync.dma_start(coso[:], cosO[:])
                nc.sync.dma_start(sino[:], sinO[:])
                nc.sync.dma_start(cosf[:], cosF[:])
                nc.sync.dma_start(sinf[:], sinF[:])
                nc.sync.dma_start(xrs[:], xres[:])
                nc.sync.dma_start(xrs2[:], xres2[:])
                nc.sync.dma_start(gat[:], g_attn[:])
                nc.sync.dma_start(gml[:], g_mlp[:])

                def unit_v(ci):
                    # V token-major for the 4 token tiles of chunk ci
                    xc = xcs[ci]
                    for kt in range(4):
                        gkt = ci * 4 + kt
                        vps = pps.tile([128, KV], f32, tag="vps", bufs=1)
                        for k in range(8):
                            nc.tensor.matmul(vps[:], xc[:, k, kt * 128:(kt + 1) * 128],
                                             wvs[:, k, :], start=(k == 0), stop=(k == 7))
                        nc.scalar.activation(
                            v_all[:, :, gkt, 0:64],
                            vps[:].rearrange("p (g d) -> p g d", g=4), AF.Copy)

                def unit_proj(u):
                    kind = u[0]
                    if kind == "Q":
                        p = u[1]
                        wqs = pwq.tile([128, 8, 128], bf16, tag="wqs")
                        nc.sync.dma_start(wqs[:], wq[p])
                        ps = pps.tile([128, OWN], f32, tag="pps", bufs=3)
                        for k in range(8):
                            nc.tensor.matmul(ps[:], wqs[:, k, :], xqs[:, k, :],
                                             start=(k == 0), stop=(k == 7))
                    else:
                        ci, kp = u[1], u[2]
                        xc = xcs[ci]
                        ps = pps.tile([128, OWN], f32, tag="pps", bufs=3)
                        for k in range(8):
                            nc.tensor.matmul(ps[:], wks[:, k, kp * 128:(kp + 1) * 128],
                                             xc[:, k, :], start=(k == 0), stop=(k == 7))
                    sq = psb.tile([128, OWN], bf16, tag="sq")
                    nc.scalar.activation(sq[:], ps[:], AF.Square)
                    return (u, ps, sq)

                def unit_norm(st):
                    u, ps, sq = st
                    inv = psb.tile([2, 2, OWN], INV_DT, tag="inv")
                    for half in range(2):
                        hs = slice(half * 64, (half + 1) * 64)
                        ss = pss.tile([2, OWN], f32, tag="ss")
                        nc.tensor.matmul(ss[:], onesc[hs, 0:2], sq[hs, :],
                                         start=True, stop=True,
                                         tile_position=(half * 64, 0))
                        rsqrt_into(psb, inv[0:2, half, :], ss[0:2, :],
                                   epsq[0:2, :], "n")
                    rep = psb.tile([128, 2, OWN], INV_DT, tag="rep")
                    nc.gpsimd.partition_broadcast(rep[:], inv[0:1, :, :],
                                                  channels=128)
                    if u[0] == "Q":
                        xn = pqn.tile([128, OWN], bf16, tag="qn", bufs=8)
                        for half in range(2):
                            hs = slice(half * 64, (half + 1) * 64)
                            nc.vector.scalar_tensor_tensor(
                                xn[hs, :], ps[hs, :], gqt[hs, u[1], None],
                                rep[hs, half, :], ALU.mult, ALU.mult)
                    else:
                        xn = pqn.tile([128, OWN], bf16, tag="kn", bufs=4)
                        for half in range(2):
                            hs = slice(half * 64, (half + 1) * 64)
                            nc.vector.tensor_tensor(xn[hs, :], ps[hs, :],
                                                    rep[hs, half, :], ALU.mult)
                    return (u, xn)

                def unit_rope(st):
                    u, xn = st
                    rot = prt.tile([128, OWN], f32, tag="rot")
                    nc.tensor.matmul(rot[:], rmt[:], xn[:], start=True, stop=True)
                    t1 = psb.tile([128, OWN], bf16, tag="t1")
                    t2 = psb.tile([128, OWN], bf16, tag="t2")
                    if u[0] == "Q":
                        nc.vector.tensor_tensor(t1[:], xn[:], coso[:], ALU.mult)
                        nc.vector.tensor_tensor(t2[:], rot[:], sino[:], ALU.mult)
                        nc.vector.tensor_tensor(qT[:, u[1], :], t1[:], t2[:],
                                                ALU.add)
                    else:
                        ci, kp = u[1], u[2]
                        sl = slice(ci * 512, (ci + 1) * 512)
                        nc.vector.tensor_tensor(t1[:], xn[:], cosf[:, sl], ALU.mult)
                        nc.vector.tensor_tensor(t2[:], rot[:], sinf[:, sl], ALU.mult)
                        nc.vector.tensor_tensor(kT[:, kp, sl], t1[:], t2[:],
                                                ALU.add)

                units = []
                qp = 0
                for ci in range(4):
                    units.append(("V", ci))
                    units.append(("K", ci, 0))
                    units.append(("K", ci, 1))
                    units.append(("Q", qp)); qp += 1
                    units.append(("Q", qp)); qp += 1

                normq, ropeq = [], []
                for u in units:
                    if u[0] == "V":
                        ci = u[1]
                        if ci + 1 < 4:
                            xcn = pxc.tile([128, 8, 512], bf16, tag="xc")
                            nc.sync.dma_start(xcn[:],
                                              xT[:, :, (ci + 1) * 512:(ci + 2) * 512])
                            xcs[ci + 1] = xcn
                        unit_v(ci)
                        continue
                    normq.append(unit_proj(u))
                    if len(normq) >= 2:
                        ropeq.append(unit_norm(normq.pop(0)))
                    if len(ropeq) >= 3:
                        unit_rope(ropeq.pop(0))
                while normq:
                    ropeq.append(unit_norm(normq.pop(0)))
                while ropeq:
                    unit_rope(ropeq.pop(0))

        # ------------- Phase D: attention ---------------------------------
        if ph_on("d"):
            with tc.tile_pool(name="pd_m", bufs=1) as pd_m, \
                 tc.tile_pool(name="pd_pt", bufs=6) as pd_pt, \
                 tc.tile_pool(name="pd_sb", bufs=3) as pd_sb, \
                 tc.tile_pool(name="pd_s", bufs=2, space="PSUM") as pd_s, \
                 tc.tile_pool(name="pd_y", bufs=2, space="PSUM") as pd_y:
                masks = pd_m.tile([128, 4, 4, 128], bf16, tag="masks")
                nc.sync.dma_start(masks[:], maskM[:])
                for t in range(4):
                    qsl = slice(t * 128, (t + 1) * 128)
                    n_chunks = t + 1
                    n_kvt = 4 * n_chunks
                    for half in range(2):
                        gA, gB = 2 * half, 2 * half + 1
                        yA = pd_y.tile([66, 4, 128], f32, tag="yA")
                        yB = pd_y.tile([66, 4, 128], f32, tag="yB")
                        qsA = qT[0:64, 4 * half:4 * half + 4, qsl]
                        qsB = qT[64:128, 4 * half:4 * half + 4, qsl]
                        av_pend = []

                        def emit_av(c, i2, ptA, ptB):
                            for isub in range(2):
                                kvt = 4 * c + 2 * i2 + isub
                                nc.tensor.matmul(yA[:], v_all[:, gA, kvt, :],
                                                 ptA[:, isub, :, :],
                                                 start=(kvt == 0),
                                                 stop=(kvt == n_kvt - 1))
                                nc.tensor.matmul(yB[:], v_all[:, gB, kvt, :],
                                                 ptB[:, isub, :, :],
                                                 start=(kvt == 0),
                                                 stop=(kvt == n_kvt - 1))

                        for c in range(n_chunks):
                            for i2 in range(2):
                                sA = pd_s.tile([128, 2, 4, 128], f32, tag="s")
                                sB = pd_s.tile([128, 2, 4, 128], f32, tag="s")
                                for isub in range(2):
                                    i = 2 * i2 + isub
                                    ks = slice((4 * c + i) * 128,
                                               (4 * c + i + 1) * 128)
                                    nc.tensor.matmul(sA[:, isub, :, :],
                                                     kT[0:64, half, ks], qsA,
                                                     start=True, stop=True,
                                                     tile_position=(0, 0))
                                    nc.tensor.matmul(sB[:, isub, :, :],
                                                     kT[64:128, half, ks], qsB,
                                                     start=True, stop=True,
                                                     tile_position=(64, 0))
                                ptA = pd_pt.tile([128, 2, 4, 128], bf16, tag="ptA")
                                ptB = pd_pt.tile([128, 2, 4, 128], bf16, tag="ptB")
                                nc.scalar.activation(ptA[:], sA[:], AF.Exp,
                                                     scale=0.125)
                                nc.scalar.activation(ptB[:], sB[:], AF.Exp,
                                                     scale=0.125)
                                if c == t:
                                    mbc = masks[:, t, 2 * i2:2 * i2 + 2, None, :] \
                                        .to_broadcast([128, 2, 4, 128])
                                    nc.vector.tensor_tensor(ptA[:], ptA[:], mbc,
                                                            ALU.mult)
                                    nc.vector.tensor_tensor(ptB[:], ptB[:], mbc,
                                                            ALU.mult)
                                av_pend.append((c, i2, ptA, ptB))
                                if len(av_pend) > 1:
                                    emit_av(*av_pend.pop(0))
                        while av_pend:
                            emit_av(*av_pend.pop(0))
                        for ab, y in ((0, yA), (1, yB)):
                            dn = pd_sb.tile([2, 4, 128], f32, tag="dn")
                            nc.vector.tensor_copy(dn[0:2, :, :], y[64:66, :, :])
                            invy = pd_sb.tile([2, 4, 128], f32, tag="invy")
                            nc.vector.reciprocal_approx_fast(
                                out=invy[:], in_=dn[:])
                            repy = pd_sb.tile([128, 4, 128], f32, tag="repy")
                            nc.gpsimd.partition_broadcast(
                                repy[:], invy[0:1, :, :], channels=128)
                            c20 = 4 * half + 2 * ab
                            y4 = y.rearrange("p (a b) q -> p a b q", b=2)
                            r4 = repy.rearrange("p (a b) q -> p a b q", b=2)
                            for ph2 in range(2):
                                nc.vector.tensor_tensor(
                                    y_all[ph2 * 64:ph2 * 64 + 64,
                                          c20:c20 + 2, qsl],
                                    y4[0:64, :, ph2, :],
                                    r4[0:64, :, ph2, :], ALU.mult)

        # ------------- Phase E: Wo + post-norm + residual -----------------
        if ph_on("e"):
            xpr = big.tile([128, 8, OWN], bf16, tag="qT_xpr")
            xpb = big.tile([128, 8, OWN], f32, tag="xpb")
            with tc.tile_pool(name="pe_sb", bufs=2) as pe_sb, \
                 tc.tile_pool(name="pe_ao", bufs=1) as pe_ao, \
                 tc.tile_pool(name="pe_w", bufs=3) as pe_w, \
                 tc.tile_pool(name="pe_ps", bufs=3, space="PSUM") as pe_ps, \
                 tc.tile_pool(name="pe_ps1", bufs=1, space="PSUM") as pe_ps1:
                ao = pe_ao.tile([128, 8, OWN], f32, tag="ao")
                ssa = pe_ps1.tile([2, OWN], f32, tag="ssa")
                a2l = []
                for o in range(8):
                    wos = pe_w.tile([128, 8, 128], bf16, tag="wos")
                    nc.sync.dma_start(wos[:], wo[o])
                    aps = pe_ps.tile([128, OWN], f32, tag="aps")
                    for k in range(8):
                        nc.tensor.matmul(aps[:], wos[:, k, :], y_all[:, k, :],
                                         start=(k == 0), stop=(k == 7))
                    nc.scalar.activation(ao[:, o, :], aps[:], AF.Copy)
                    a2 = pe_sb.tile([128, OWN], bf16, tag="a2", bufs=3)
                    nc.scalar.activation(a2[:], aps[:], AF.Square)
                    a2l.append(a2)
                    if o >= 1:
                        nc.tensor.matmul(ssa[:], onesc[:, 0:2], a2l[o - 1][:],
                                         start=(o == 1), stop=False)
                nc.tensor.matmul(ssa[:], onesc[:, 0:2], a2l[7][:],
                                 start=False, stop=True)
                inva = pe_sb.tile([2, OWN], INV_DT, tag="inva")
                if USE_ABSRSQRT:
                    nc.scalar.activation(inva[:], ssa[0:2, :],
                                         AF.Abs_reciprocal_sqrt,
                                         scale=1.0 / D, bias=eps6[0:2, :])
                else:
                    rmsa = pe_sb.tile([2, OWN], f32, tag="rmsa")
                    nc.scalar.activation(rmsa[:], ssa[0:2, :], AF.Sqrt,
                                         scale=1.0 / D, bias=eps6[0:2, :])
                    nc.vector.reciprocal_approx_fast(out=inva[:], in_=rmsa[:])
                repa = pe_sb.tile([128, OWN], INV_DT, tag="repa")
                nc.gpsimd.partition_broadcast(repa[:], inva[0:1, :], channels=128)
                tmps = []
                for o in range(8):
                    tmp = pe_sb.tile([128, OWN], f32, tag="tmpe", bufs=8)
                    nc.vector.scalar_tensor_tensor(
                        tmp[:], ao[:, o, :], gat[:, o, None], repa[:],
                        ALU.mult, ALU.mult)
                    nc.vector.tensor_tensor(xpr[:, o, :], tmp[:], xrs[:, o, :],
                                            ALU.add)
                    tmps.append(tmp)
                for o in range(8):
                    nc.vector.tensor_tensor(xpb[:, o, :], tmps[o][:],
                                            xrs2[:, o, :], ALU.add)

        # ------------- Phase F: MLP ---------------------------------------
        if ph_on("f"):
            mout = big.tile([128, 8, OWN], f32, tag="xrs2_mout")
            with tc.tile_pool(name="pf_h2", bufs=1) as pf_h2, \
                 tc.tile_pool(name="pf_sb", bufs=3) as pf_sb, \
                 tc.tile_pool(name="pf_wf", bufs=3) as pf_wf, \
                 tc.tile_pool(name="pf_wp", bufs=3) as pf_wp, \
                 tc.tile_pool(name="pf_ps", bufs=2, space="PSUM") as pf_ps, \
                 tc.tile_pool(name="pf_mo", bufs=1, space="PSUM") as pf_mo, \
                 tc.tile_pool(name="pf_ss", bufs=1, space="PSUM") as pf_ss:
                h2 = pf_h2.tile([128, 32, OWN], bf16, tag="h2")
                ssm = pf_ss.tile([2, OWN], f32, tag="ssm")

                def emit_fc(hc):
                    wfs = pf_wf.tile([128, 8, 128], bf16, tag="wfs")
                    nc.sync.dma_start(wfs[:], wfc[hc])
                    hps = pf_ps.tile([128, OWN], f32, tag="hps")
                    for k in range(8):
                        nc.tensor.matmul(hps[:], wfs[:, k, :], xpr[:, k, :],
                                         start=(k == 0), stop=(k == 7))
                    hr = pf_sb.tile([128, OWN], bf16, tag="hr")
                    nc.scalar.activation(hr[:], hps[:], AF.Relu)
                    nc.vector.tensor_tensor(h2[:, hc, :], hr[:], hr[:], ALU.mult)

                mo_all = []
                for ohalf in range(2):
                    mo_ps = [pf_mo.tile([128, OWN], f32, name=f"mo{oi}",
                                        tag=f"mo{oi}") for oi in range(4)]
                    mo_all.append(mo_ps)

                for hc in range(34):
                    if hc < 32:
                        emit_fc(hc)
                    if hc >= 2:
                        hp = hc - 2
                        wps = pf_wp.tile([128, 4, 128], bf16, tag="wps")
                        nc.sync.dma_start(wps[:], wproj[hp, 0])
                        for oi in range(4):
                            nc.tensor.matmul(mo_all[0][oi][:], wps[:, oi, :],
                                             h2[:, hp, :],
                                             start=(hp == 0), stop=(hp == 31))
                # ohalf0 copies/squares run on scalar during the proj1 matmuls
                m2l = []
                for oi in range(4):
                    nc.scalar.activation(mout[:, oi, :], mo_all[0][oi][:],
                                         AF.Copy)
                    m2 = pf_sb.tile([128, OWN], bf16, tag="m2", bufs=8)
                    nc.scalar.activation(m2[:], mo_all[0][oi][:], AF.Square)
                    m2l.append(m2)
                for hc in range(32):
                    wps = pf_wp.tile([128, 4, 128], bf16, tag="wps")
                    nc.sync.dma_start(wps[:], wproj[hc, 1])
                    for oi in range(4):
                        nc.tensor.matmul(mo_all[1][oi][:], wps[:, oi, :],
                                         h2[:, hc, :],
                                         start=(hc == 0), stop=(hc == 31))
                    if hc < 4:
                        nc.tensor.matmul(ssm[:], onesc[:, 0:2], m2l[hc][:],
                                         start=(hc == 0), stop=False)
                for oi in range(4):
                    m2 = pf_sb.tile([128, OWN], bf16, tag="m2", bufs=8)
                    nc.scalar.activation(m2[:], mo_all[1][oi][:], AF.Square)
                    m2l.append(m2)
                for o in range(4, 8):
                    nc.tensor.matmul(ssm[:], onesc[:, 0:2], m2l[o][:],
                                     start=False, stop=(o == 7))
                invm = pf_sb.tile([2, OWN], INV_DT, tag="invm")
                if USE_ABSRSQRT:
                    nc.scalar.activation(invm[:], ssm[0:2, :],
                                         AF.Abs_reciprocal_sqrt,
                                         scale=1.0 / D, bias=eps6[0:2, :])
                else:
                    rmsm = pf_sb.tile([2, OWN], f32, tag="rmsm")
                    nc.scalar.activation(rmsm[:], ssm[0:2, :], AF.Sqrt,
                                         scale=1.0 / D, bias=eps6[0:2, :])
                    nc.vector.reciprocal_approx_fast(out=invm[:], in_=rmsm[:])
                repm = pf_sb.tile([128, OWN], INV_DT, tag="repm")
                nc.gpsimd.partition_broadcast(repm[:], invm[0:1, :], channels=128)
                for o in range(8):
                    msrc = mout[:, o, :] if o < 4 else mo_all[1][o - 4][:]
                    tmp = pf_sb.tile([128, OWN], f32, tag="tmpf")
                    nc.vector.scalar_tensor_tensor(
                        tmp[:], msrc, gml[:, o, None], repm[:],
                        ALU.mult, ALU.mult)
                    outv = pf_sb.tile([128, OWN], f32, tag="outv")
                    nc.vector.tensor_tensor(outv[:], tmp[:], xpb[:, o, :],
                                            ALU.add)
                    nc.sync.dma_start(out_t[:, o, :], outv[:])

        rope_stack.close()

    nc.finalize()
    return nc


def _feat_major(a):
    """[F, T] -> device layout [128, F//128, T]."""
    F, T = a.shape
    return np.ascontiguousarray(a.reshape(F // 128, 128, T).transpose(1, 0, 2))


def _vec_dev(v):
    return np.ascontiguousarray(v.reshape(-1, 128).T)


def _bf(a):
    return np.ascontiguousarray(np.asarray(a, np.float32)).astype(ml_dtypes.bfloat16)


_CACHE = {}
_RUN_KW = {}



def kernel(x, attn_norm_w, mlp_norm_w, attn_post_norm_w, mlp_post_norm_w,
           attn_scale, mlp_scale, attn_mod_gain, attn_mod_bias,
           mlp_mod_gain, mlp_mod_bias, Wq, Wk, Wv, Wo, q_gain, fc_w, proj_w):
    x = np.asarray(x, np.float32)
    q_gain = np.asarray(q_gain, np.float32)

    if "nc" not in _CACHE:
        _CACHE["nc"] = build()
    nc = _CACHE["nc"]

    anw = np.asarray(attn_norm_w, np.float32)
    mnw = np.asarray(mlp_norm_w, np.float32)
    fc_eff = np.asarray(fc_w, np.float32) * mnw[None, :]

    # host-side input rmsnorm: xn = x * anw / rms1(x)
    ms1 = np.mean(np.square(x), axis=-1, keepdims=True)
    xn = (x * (1.0 / np.sqrt(ms1 + EPS_BLOCK))) * anw[None, None, :]

    # Wq columns permuted so p-tile p holds heads PAIRS[p] stacked (64+64)
    perm = np.zeros(D, np.int64)
    for p, (a, b) in enumerate(PAIRS):
        perm[p * 128:p * 128 + 64] = np.arange(a * 64, a * 64 + 64)
        perm[p * 128 + 64:(p + 1) * 128] = np.arange(b * 64, b * 64 + 64)
    WqTp = np.asarray(Wq, np.float32).T[:, perm]
    wq_dev = np.stack([_feat_major(WqTp[:, p * 128:(p + 1) * 128]) for p in range(8)])
    wk_dev = _feat_major(np.asarray(Wk, np.float32).T)
    wv_dev = _feat_major(np.asarray(Wv, np.float32).T)

    # Wo rows permuted to match y_all layout: chunk c2 = 4*half + 2*ab + i//2,
    # partition ph2*64+f  ->  original feature 64*(4*(2*half+ab) + i) + f,
    # with i = 2*(c2 % 2) + ph2.
    perm2 = np.zeros(D, np.int64)
    for c2 in range(8):
        halfg = c2 // 2          # kv-head index (2*half + ab)
        for ph2 in range(2):
            i = 2 * (c2 % 2) + ph2
            h_orig = 4 * halfg + i
            rows = np.arange(64)
            perm2[c2 * 128 + ph2 * 64 + rows] = 64 * h_orig + rows
    WoT = np.asarray(Wo, np.float32).T[perm2, :]
    wo_dev = np.stack([_feat_major(WoT[:, o * 128:(o + 1) * 128]) for o in range(8)])

    fcT = fc_eff.T
    wfc_dev = np.stack([_feat_major(fcT[:, h * 128:(h + 1) * 128]) for h in range(32)])
    projT = np.asarray(proj_w, np.float32).T                  # [4096, 1024]
    wproj_dev = np.ascontiguousarray(
        projT.reshape(32, 128, 2, 4, 128).transpose(0, 2, 1, 3, 4))

    # rope tables; sin sign-folded: x1-groups (even 32-blocks) get -sin
    inv_freq = 1.0 / (ROPE_BASE ** (np.arange(0, HD, 2, dtype=np.float32) / HD))
    tpos = np.arange(S, dtype=np.float32)
    freqs = np.outer(tpos, inv_freq).astype(np.float32)
    cosT = np.ascontiguousarray(np.tile(np.cos(freqs).T, (4, 1)))   # [128, S]
    sinN = np.ascontiguousarray(np.tile(np.sin(freqs).T, (4, 1)))
    # rope rotate-half matrix with signs: rot = R @ x (per 64-feature head)
    R = np.zeros((128, 128), np.float32)
    for p in range(128):
        if p % 64 < 32:
            R[p, p + 32] = 1.0
        else:
            R[p, p - 32] = -1.0
    rmat_h = np.ascontiguousarray(R.T)

    gat_v = (np.asarray(attn_post_norm_w, np.float32)
             * np.asarray(attn_mod_gain, np.float32)
             * np.asarray(attn_scale, np.float32))
    bat_v = np.asarray(attn_mod_bias, np.float32) * np.asarray(attn_scale, np.float32)
    gml_v = (np.asarray(mlp_post_norm_w, np.float32)
             * np.asarray(mlp_mod_gain, np.float32)
             * np.asarray(mlp_scale, np.float32))
    bml_v = np.asarray(mlp_mod_bias, np.float32) * np.asarray(mlp_scale, np.float32)

    gq_h = np.zeros((128, 8), np.float32)
    for p, (a, b) in enumerate(PAIRS):
        gq_h[0:64, p] = q_gain[a]
        gq_h[64:128, p] = q_gain[b]

    shared = {
        "wq": _bf(wq_dev), "wk": _bf(wk_dev), "wv": _bf(wv_dev),
        "wo": _bf(wo_dev), "wfc": _bf(wfc_dev), "wproj": _bf(wproj_dev),
        "cosF": _bf(cosT), "sinF": _bf(sinN), "rmat": _bf(rmat_h),
        "ones_c": _bf(np.ones((128, 2), np.float32)),
        "gq": gq_h,
        "g_attn": _vec_dev(gat_v), "g_mlp": _vec_dev(gml_v),
    }

    in_maps = []
    owners = []
    for c in range(8):
        b, j = c // 4, c % 4
        rows = np.concatenate(
            [np.arange((j + 4 * t) * 128, (j + 4 * t + 1) * 128) for t in range(4)])
        owners.append((b, rows))
        xnb = xn[b].T
        x_own_raw = x[b].T[:, rows]
        mask = np.zeros((4, 4, 128, 128), np.float32)
        for t in range(4):
            m = j + 4 * t
            q_idx = m * 128 + np.arange(128)
            for ktl in range(4):
                kv_idx = 512 * t + 128 * ktl + np.arange(128)
                mask[t, ktl] = (kv_idx[:, None] <= q_idx[None, :])
        m_in = {
            "xT": _bf(_feat_major(xnb)),
            "xq": _bf(_feat_major(xnb[:, rows])),
            "xres": _feat_major(x_own_raw + bat_v[:, None]),
            "xres2": _feat_major(x_own_raw + (bat_v + bml_v)[:, None]),
            "cosO": _bf(cosT[:, rows]),
            "sinO": _bf(sinN[:, rows]),
            "maskM": _bf(np.ascontiguousarray(mask.transpose(2, 0, 1, 3))),
        }
        m_in.update(shared)
        in_maps.append(m_in)

    res = run_bass_kernel_spmd(nc, in_maps, core_ids=list(range(8)),
                               **_RUN_KW)
    _CACHE["last_result"] = res

    out = np.empty((B, S, D), np.float32)
    for c in range(8):
        b, rows = owners[c]
        o = res.results[c]["out"]
        out[b, rows, :] = o.transpose(2, 1, 0).reshape(OWN, D)
    return out


# revision 22
# speedup vs baseline: 1.7135x; 1.0946x over previous
"""Trainium2 Bass kernel for one dense transformer block (B=2, S=2048, D=1024,
16 q-heads / 4 kv-heads GQA, squared-ReLU MLP), data-parallel over 8 NeuronCores.

Sharding: core c = (b, j), b = c // 4, j = c % 4, owns q-token tiles
{j, j+4, j+8, j+12} (128 tokens each) of batch b. K/V are computed for the full
sequence on every core (no collectives). The kv range for own q-tile t is
padded to 512*(t+1); causality enforced with per-core 0/1 masks on the
diagonal 512-wide kv chunk.

v2: bf16 matmul datapath (fp32 PSUM accumulation), host-side input rmsnorm
(xn = x*attn_norm_w/rms1 fed from DRAM), Q/K/V emission interleaved with
lagged norm + rope stages, rope rotate-half via a PE matmul with signs baked
into the R matrix, per-head normalizers replicated with gpsimd
partition_broadcast (q_gain folded into a per-partition stt scalar),
Abs_reciprocal_sqrt / reciprocal_approx_fast for all normalizers, exps
batched to 1024 elem/partition, AV matmuls lagged one tile behind QK+exp,
software-pipelined fc->proj MLP with the post-norm tail overlapped.

Numerical identities (exact up to negligible eps rescaling):
  - per-head q/k rmsnorm is scale-invariant per token -> Q/K project from the
    host-normalized xn without correction
  - the MLP input rmsnorm cancels through relu()^2 -> proj -> post-rmsnorm
  - no softmax max-subtraction (logits bounded by |q||k|/8 = 8)
  - softmax denominator = ones-columns appended to V in the AV matmul
"""

import os

import numpy as np
import ml_dtypes

import concourse.bass as bass
from concourse import bacc
import concourse.tile as tile
import concourse.mybir as mybir
from concourse.bass_utils import run_bass_kernel_spmd

f32 = mybir.dt.float32
f32r = mybir.dt.float32r
bf16 = mybir.dt.bfloat16
AF = mybir.ActivationFunctionType
ALU = mybir.AluOpType

B, S, D = 2, 2048, 1024
H, HKV, HD = 16, 4, 64
MLP_HID = 4 * D
KV = HKV * HD
NT = 16
OWN = 512
EPS_BLOCK = 1e-6
EPS_QK = float(np.finfo(np.float32).eps)
ROPE_BASE = 10000.0

PAIRS = [(0, 4), (1, 5), (2, 6), (3, 7), (8, 12), (9, 13), (10, 14), (11, 15)]

USE_ABSRSQRT = os.environ.get("KERNEL_ABSRSQRT", "1") == "1"
INV_DT = bf16 if USE_ABSRSQRT else f32

PHASE_ORDER = ["c", "ab", "d", "e", "f"]


def build():
    max_ph = os.environ.get("KERNEL_PHASES", "f")
    ph_on = lambda p: PHASE_ORDER.index(p) <= PHASE_ORDER.index(max_ph)
    bacc.Bacc.move_matmul_waits_to_ldweights = lambda self: None
    nc = bacc.Bacc(None)

    def dram_in(name, shape, dt=bf16):
        return nc.dram_tensor(name, list(shape), dt, kind="ExternalInput")

    xT = dram_in("xT", (128, 8, S))              # normalized x, feature-major
    xq = dram_in("xq", (128, 8, OWN))            # normalized own x
    xres = dram_in("xres", (128, 8, OWN), f32)   # raw own x + attn bias
    xres2 = dram_in("xres2", (128, 8, OWN), f32) # xres + mlp bias
    wq = dram_in("wq", (8, 128, 8, 128))
    wk = dram_in("wk", (128, 8, KV))
    wv = dram_in("wv", (128, 8, KV))
    wo = dram_in("wo", (8, 128, 8, 128))
    wfc = dram_in("wfc", (32, 128, 8, 128))
    wproj = dram_in("wproj", (32, 2, 128, 4, 128))
    cosF = dram_in("cosF", (128, S))
    sinF = dram_in("sinF", (128, S))
    cosO = dram_in("cosO", (128, OWN))
    sinO = dram_in("sinO", (128, OWN))
    rmat = dram_in("rmat", (128, 128))           # rope rotate-half (signs baked)
    maskM = dram_in("maskM", (128, 4, 4, 128))
    ones_c = dram_in("ones_c", (128, 2))
    gq = dram_in("gq", (128, 8), f32)            # per-p-tile head gains
    g_attn = dram_in("g_attn", (128, 8), f32)
    g_mlp = dram_in("g_mlp", (128, 8), f32)

    out_t = nc.dram_tensor("out", [128, 8, OWN], f32, kind="ExternalOutput")

    def rsqrt_into(pool, out_ap, in_ap, bias_ap, pfx):
        """out_ap = 1/sqrt(in_ap/HD + eps); in_ap [2, n] psum."""
        if USE_ABSRSQRT:
            nc.scalar.activation(out_ap, in_ap, AF.Abs_reciprocal_sqrt,
                                 scale=1.0 / HD, bias=bias_ap)
        else:
            rms = pool.tile([2, in_ap.shape[-1]], f32, name=f"{pfx}rms",
                            tag=f"{pfx}rms")
            nc.scalar.activation(rms[:], in_ap, AF.Sqrt, scale=1.0 / HD,
                                 bias=bias_ap)
            nc.vector.reciprocal_approx_fast(out=out_ap, in_=rms[:])

    with tile.TileContext(nc) as tc, \
         tc.tile_pool(name="cst", bufs=1) as cst, \
         tc.tile_pool(name="big", bufs=1) as big:
        # --- constants / tables -------------------------------------------
        onesc = cst.tile([128, 2], bf16, tag="onesc")
        nc.sync.dma_start(onesc[:], ones_c[:])
        gqt = cst.tile([128, 8], f32, tag="gqt")
        nc.sync.dma_start(gqt[:], gq[:])
        rmt = cst.tile([128, 128], bf16, tag="rmt")
        epsq = cst.tile([128, 1], f32, tag="epsq")
        nc.vector.memset(epsq[:], EPS_QK)
        eps6 = cst.tile([128, 1], f32, tag="eps6")
        nc.vector.memset(eps6[:], EPS_BLOCK)
        gat = cst.tile([128, 8], f32, tag="gat")
        gml = cst.tile([128, 8], f32, tag="gml")

        from contextlib import ExitStack
        rope_stack = ExitStack()
        ropep = rope_stack.enter_context(tc.tile_pool(name="ropep", bufs=1))
        coso = ropep.tile([128, OWN], bf16, tag="coso")
        sino = ropep.tile([128, OWN], bf16, tag="sino")
        cosf = ropep.tile([128, S], bf16, tag="cosf")
        sinf = ropep.tile([128, S], bf16, tag="sinf")

        kT = big.tile([128, 2, S], bf16, tag="kT")
        v_all = big.tile([128, 4, NT, 66], bf16, tag="v_all")
        qT = big.tile([128, 8, OWN], bf16, tag="qT_xpr")
        y_all = big.tile([128, 8, OWN], bf16, tag="yall")
        xrs = big.tile([128, 8, OWN], f32, tag="xrs")
        xrs2 = big.tile([128, 8, OWN], f32, tag="xrs2_mout")

        # ------------- Phases C+AB interleaved: Q, K, V -------------------
        if ph_on("c"):
            nc.vector.tensor_copy(
                v_all[:, :, :, 64:66],
                onesc[:, 0, None, None].to_broadcast([128, 4, NT, 2]))
            with tc.tile_pool(name="px", bufs=1) as px, \
                 tc.tile_pool(name="pxc", bufs=2) as pxc, \
                 tc.tile_pool(name="pw", bufs=1) as pw, \
                 tc.tile_pool(name="pwq", bufs=3) as pwq, \
                 tc.tile_pool(name="psb", bufs=3) as psb, \
                 tc.tile_pool(name="pqn", bufs=1) as pqn, \
                 tc.tile_pool(name="pps", bufs=1, space="PSUM") as pps, \
                 tc.tile_pool(name="pss", bufs=2, space="PSUM") as pss, \
                 tc.tile_pool(name="prt", bufs=2, space="PSUM") as prt:
                # critical-path DMAs first: first chunk + V/K weights + xq
                xcs = {}
                xc0 = pxc.tile([128, 8, 512], bf16, tag="xc")
                nc.sync.dma_start(xc0[:], xT[:, :, 0:512])
                xcs[0] = xc0
                wvs = pw.tile([128, 8, KV], bf16, tag="wvs")
                nc.sync.dma_start(wvs[:], wv[:])
                wks = pw.tile([128, 8, KV], bf16, tag="wks")
                nc.sync.dma_start(wks[:], wk[:])
                xqs = px.tile([128, 8, OWN], bf16, tag="xqs")
                nc.sync.dma_start(xqs[:], xq[:])
                # long-lead tables and residuals after the critical batch
                nc.sync.dma_start(rmt[:], rmat[:])
                nc.sync.dma_start(coso[:], cosO[:])
                nc.sync.dma_start(sino[:], sinO[:])
                nc.sync.dma_start(cosf[:], cosF[:])
                nc.sync.dma_start(sinf[:], sinF[:])
                nc.sync.dma_start(xrs[:], xres[:])
                nc.sync.dma_start(xrs2[:], xres2[:])
                nc.sync.dma_start(gat[:], g_attn[:])
                nc.sync.dma_start(gml[:], g_mlp[:])

                def unit_v(ci):
                    # V token-major for the 4 token tiles of chunk ci
                    xc = xcs[ci]
                    for kt in range(4):
                        gkt = ci * 4 + kt
                        vps = pps.tile([128, KV], f32, tag="vps", bufs=1)
                        for k in range(8):
                            nc.tensor.matmul(vps[:], xc[:, k, kt * 128:(kt + 1) * 128],
                                             wvs[:, k, :], start=(k == 0), stop=(k == 7))
                        nc.scalar.activation(
                            v_all[:, :, gkt, 0:64],
                            vps[:].rearrange("p (g d) -> p g d", g=4), AF.Copy)

                def unit_proj(u):
                    kind = u[0]
                    if kind == "Q":
                        p = u[1]
                        wqs = pwq.tile([128, 8, 128], bf16, tag="wqs")
                        nc.sync.dma_start(wqs[:], wq[p])
                        ps = pps.tile([128, OWN], f32, tag="pps", bufs=3)
                        for k in range(8):
                            nc.tensor.matmul(ps[:], wqs[:, k, :], xqs[:, k, :],
                                             start=(k == 0), stop=(k == 7))
                    else:
                        ci, kp = u[1], u[2]
                        xc = xcs[ci]
                        ps = pps.tile([128, OWN], f32, tag="pps", bufs=3)
                        for k in range(8):
                            nc.tensor.matmul(ps[:], wks[:, k, kp * 128:(kp + 1) * 128],
                                             xc[:, k, :], start=(k == 0), stop=(k == 7))
                    sq = psb.tile([128, OWN], bf16, tag="sq")
                    nc.scalar.activation(sq[:], ps[:], AF.Square)
                    return (u, ps, sq)

                def unit_norm(st):
                    u, ps, sq = st
                    inv = psb.tile([2, 2, OWN], INV_DT, tag="inv")
                    for half in range(2):
                        hs = slice(half * 64, (half + 1) * 64)
                        ss = pss.tile([2, OWN], f32, tag="ss")
                        nc.tensor.matmul(ss[:], onesc[hs, 0:2], sq[hs, :],
                                         start=True, stop=True,
                                         tile_position=(half * 64, 0))
                        rsqrt_into(psb, inv[0:2, half, :], ss[0:2, :],
                                   epsq[0:2, :], "n")
                    rep = psb.tile([128, 2, OWN], INV_DT, tag="rep")
                    nc.gpsimd.partition_broadcast(rep[:], inv[0:1, :, :],
                                                  channels=128)
                    if u[0] == "Q":
                        xn = pqn.tile([128, OWN], bf16, tag="qn", bufs=8)
                        for half in range(2):
                            hs = slice(half * 64, (half + 1) * 64)
                            nc.vector.scalar_tensor_tensor(
                                xn[hs, :], ps[hs, :], gqt[hs, u[1], None],
                                rep[hs, half, :], ALU.mult, ALU.mult)
                    else:
                        xn = pqn.tile([128, OWN], bf16, tag="kn", bufs=4)
                        for half in range(2):
                            hs = slice(half * 64, (half + 1) * 64)
                            nc.vector.tensor_tensor(xn[hs, :], ps[hs, :],
                                                    rep[hs, half, :], ALU.mult)
                    return (u, xn)

                def unit_rope(st):
                    u, xn = st
                    rot = prt.tile([128, OWN], f32, tag="rot")
                    nc.tensor.matmul(rot[:], rmt[:], xn[:], start=True, stop=True)
                    t1 = psb.tile([128, OWN], bf16, tag="t1")
                    t2 = psb.tile([128, OWN], bf16, tag="t2")
                    if u[0] == "Q":
                        nc.vector.tensor_tensor(t1[:], xn[:], coso[:], ALU.mult)
                        nc.vector.tensor_tensor(t2[:], rot[:], sino[:], ALU.mult)
                        nc.vector.tensor_tensor(qT[:, u[1], :], t1[:], t2[:],
                                                ALU.add)
                    else:
                        ci, kp = u[1], u[2]
                        sl = slice(ci * 512, (ci + 1) * 512)
                        nc.vector.tensor_tensor(t1[:], xn[:], cosf[:, sl], ALU.mult)
                        nc.vector.tensor_tensor(t2[:], rot[:], sinf[:, sl], ALU.mult)
                        nc.vector.tensor_tensor(kT[:, kp, sl], t1[:], t2[:],
                                                ALU.add)

                units = []
                qp = 0
                for ci in range(4):
                    units.append(("V", ci))
                    units.append(("K", ci, 0))
                    units.append(("K", ci, 1))
                    units.append(("Q", qp)); qp += 1
                    units.append(("Q", qp)); qp += 1

                normq, ropeq = [], []
                for u in units:
                    if u[0] == "V":
                        ci = u[1]
                        if ci + 1 < 4:
                            xcn = pxc.tile([128, 8, 512], bf16, tag="xc")
                            nc.sync.dma_start(xcn[:],
                                              xT[:, :, (ci + 1) * 512:(ci + 2) * 512])
                            xcs[ci + 1] = xcn
                        unit_v(ci)
                        continue
                    normq.append(unit_proj(u))
                    if len(normq) >= 2:
                        ropeq.append(unit_norm(normq.pop(0)))
                    if len(ropeq) >= 3:
                        unit_rope(ropeq.pop(0))
                while normq:
                    ropeq.append(unit_norm(normq.pop(0)))
                while ropeq:
                    unit_rope(ropeq.pop(0))

        # ------------- Phase D: attention ---------------------------------
        wo_stack = ExitStack()
        pe_w = wo_stack.enter_context(tc.tile_pool(name="pe_w", bufs=1))
        wosl = []
        for o in range(8):
            wos = pe_w.tile([128, 8, 128], bf16, tag=f"wos{o}")
            wosl.append(wos)
        if ph_on("d"):
            for o in range(8):
                nc.sync.dma_start(wosl[o][:], wo[o])
            with tc.tile_pool(name="pd_m", bufs=1) as pd_m, \
                 tc.tile_pool(name="pd_pt", bufs=6) as pd_pt, \
                 tc.tile_pool(name="pd_sb", bufs=3) as pd_sb, \
                 tc.tile_pool(name="pd_s", bufs=2, space="PSUM") as pd_s, \
                 tc.tile_pool(name="pd_y", bufs=2, space="PSUM") as pd_y:
                masks = pd_m.tile([128, 4, 4, 128], bf16, tag="masks")
                nc.sync.dma_start(masks[:], maskM[:])
                for t in range(4):
                    qsl = slice(t * 128, (t + 1) * 128)
                    n_chunks = t + 1
                    n_kvt = 4 * n_chunks
                    for half in range(2):
                        gA, gB = 2 * half, 2 * half + 1
                        yA = pd_y.tile([66, 4, 128], f32, tag="yA")
                        yB = pd_y.tile([66, 4, 128], f32, tag="yB")
                        qsA = qT[0:64, 4 * half:4 * half + 4, qsl]
                        qsB = qT[64:128, 4 * half:4 * half + 4, qsl]
                        av_pend = []

                        def emit_av(c, i2, ptA, ptB):
                            for isub in range(2):
                                kvt = 4 * c + 2 * i2 + isub
                                nc.tensor.matmul(yA[:], v_all[:, gA, kvt, :],
                                                 ptA[:, isub, :, :],
                                                 start=(kvt == 0),
                                                 stop=(kvt == n_kvt - 1))
                                nc.tensor.matmul(yB[:], v_all[:, gB, kvt, :],
                                                 ptB[:, isub, :, :],
                                                 start=(kvt == 0),
                                                 stop=(kvt == n_kvt - 1))

                        for c in range(n_chunks):
                            for i2 in range(2):
                                sA = pd_s.tile([128, 2, 4, 128], f32, tag="s")
                                sB = pd_s.tile([128, 2, 4, 128], f32, tag="s")
                                for isub in range(2):
                                    i = 2 * i2 + isub
                                    ks = slice((4 * c + i) * 128,
                                               (4 * c + i + 1) * 128)
                                    nc.tensor.matmul(sA[:, isub, :, :],
                                                     kT[0:64, half, ks], qsA,
                                                     start=True, stop=True,
                                                     tile_position=(0, 0))
                                    nc.tensor.matmul(sB[:, isub, :, :],
                                                     kT[64:128, half, ks], qsB,
                                                     start=True, stop=True,
                                                     tile_position=(64, 0))
                                ptA = pd_pt.tile([128, 2, 4, 128], bf16, tag="ptA")
                                ptB = pd_pt.tile([128, 2, 4, 128], bf16, tag="ptB")
                                nc.scalar.activation(ptA[:], sA[:], AF.Exp,
                                                     scale=0.125)
                                nc.scalar.activation(ptB[:], sB[:], AF.Exp,
                                                     scale=0.125)
                                if c == t:
                                    mbc = masks[:, t, 2 * i2:2 * i2 + 2, None, :] \
                                        .to_broadcast([128, 2, 4, 128])
                                    nc.vector.tensor_tensor(ptA[:], ptA[:], mbc,
                                                            ALU.mult)
                                    nc.vector.tensor_tensor(ptB[:], ptB[:], mbc,
                                                            ALU.mult)
                                av_pend.append((c, i2, ptA, ptB))
                                if len(av_pend) > 1:
                                    emit_av(*av_pend.pop(0))
                        while av_pend:
                            emit_av(*av_pend.pop(0))
                        for ab, y in ((0, yA), (1, yB)):
                            dn = pd_sb.tile([2, 4, 128], f32, tag="dn")
                            nc.vector.tensor_copy(dn[0:2, :, :], y[64:66, :, :])
                            invy = pd_sb.tile([2, 4, 128], f32, tag="invy")
                            nc.vector.reciprocal_approx_fast(
                                out=invy[:], in_=dn[:])
                            repy = pd_sb.tile([128, 4, 128], f32, tag="repy")
                            nc.gpsimd.partition_broadcast(
                                repy[:], invy[0:1, :, :], channels=128)
                            c20 = 4 * half + 2 * ab
                            y4 = y.rearrange("p (a b) q -> p a b q", b=2)
                            r4 = repy.rearrange("p (a b) q -> p a b q", b=2)
                            for ph2 in range(2):
                                nc.vector.tensor_tensor(
                                    y_all[ph2 * 64:ph2 * 64 + 64,
                                          c20:c20 + 2, qsl],
                                    y4[0:64, :, ph2, :],
                                    r4[0:64, :, ph2, :], ALU.mult)

        # ------------- Phase E: Wo + post-norm + residual -----------------
        if ph_on("e"):
            xpr = big.tile([128, 8, OWN], bf16, tag="qT_xpr")
            xpb = big.tile([128, 8, OWN], f32, tag="xpb")
            with tc.tile_pool(name="pe_sb", bufs=2) as pe_sb, \
                 tc.tile_pool(name="pe_ao", bufs=1) as pe_ao, \
                 tc.tile_pool(name="pe_ps", bufs=3, space="PSUM") as pe_ps, \
                 tc.tile_pool(name="pe_ps1", bufs=1, space="PSUM") as pe_ps1:
                ao = pe_ao.tile([128, 8, OWN], f32, tag="ao")
                ssa = pe_ps1.tile([2, OWN], f32, tag="ssa")
                a2l = []
                for o in range(8):
                    wos = wosl[o]
                    aps = pe_ps.tile([128, OWN], f32, tag="aps")
                    for k in range(8):
                        nc.tensor.matmul(aps[:], wos[:, k, :], y_all[:, k, :],
                                         start=(k == 0), stop=(k == 7))
                    nc.scalar.activation(ao[:, o, :], aps[:], AF.Copy)
                    a2 = pe_sb.tile([128, OWN], bf16, tag="a2", bufs=3)
                    nc.scalar.activation(a2[:], aps[:], AF.Square)
                    a2l.append(a2)
                    if o >= 1:
                        nc.tensor.matmul(ssa[:], onesc[:, 0:2], a2l[o - 1][:],
                                         start=(o == 1), stop=False)
                nc.tensor.matmul(ssa[:], onesc[:, 0:2], a2l[7][:],
                                 start=False, stop=True)
                inva = pe_sb.tile([2, OWN], INV_DT, tag="inva")
                if USE_ABSRSQRT:
                    nc.scalar.activation(inva[:], ssa[0:2, :],
                                         AF.Abs_reciprocal_sqrt,
                                         scale=1.0 / D, bias=eps6[0:2, :])
                else:
                    rmsa = pe_sb.tile([2, OWN], f32, tag="rmsa")
                    nc.scalar.activation(rmsa[:], ssa[0:2, :], AF.Sqrt,
                                         scale=1.0 / D, bias=eps6[0:2, :])
                    nc.vector.reciprocal_approx_fast(out=inva[:], in_=rmsa[:])
                repa = pe_sb.tile([128, OWN], INV_DT, tag="repa")
                nc.gpsimd.partition_broadcast(repa[:], inva[0:1, :], channels=128)
                tmps = []
                for o in range(8):
                    tmp = pe_sb.tile([128, OWN], f32, tag="tmpe", bufs=8)
                    nc.vector.scalar_tensor_tensor(
                        tmp[:], ao[:, o, :], gat[:, o, None], repa[:],
                        ALU.mult, ALU.mult)
                    nc.vector.tensor_tensor(xpr[:, o, :], tmp[:], xrs[:, o, :],
                                            ALU.add)
                    tmps.append(tmp)
                for o in range(8):
                    nc.gpsimd.tensor_tensor(xpb[:, o, :], tmps[o][:],
                                            xrs2[:, o, :], ALU.add)
            wo_stack.close()

        # ------------- Phase F: MLP ---------------------------------------
        if ph_on("f"):
            mout = big.tile([128, 8, OWN], f32, tag="xrs2_mout")
            with tc.tile_pool(name="pf_h2", bufs=1) as pf_h2, \
                 tc.tile_pool(name="pf_sb", bufs=3) as pf_sb, \
                 tc.tile_pool(name="pf_wf", bufs=3) as pf_wf, \
                 tc.tile_pool(name="pf_wp", bufs=3) as pf_wp, \
                 tc.tile_pool(name="pf_ps", bufs=2, space="PSUM") as pf_ps, \
                 tc.tile_pool(name="pf_mo", bufs=1, space="PSUM") as pf_mo, \
                 tc.tile_pool(name="pf_ss", bufs=1, space="PSUM") as pf_ss:
                h2 = pf_h2.tile([128, 32, OWN], bf16, tag="h2")
                ssm = pf_ss.tile([2, OWN], f32, tag="ssm")

                def emit_fc(hc):
                    wfs = pf_wf.tile([128, 8, 128], bf16, tag="wfs")
                    nc.sync.dma_start(wfs[:], wfc[hc])
                    hps = pf_ps.tile([128, OWN], f32, tag="hps")
                    for k in range(8):
                        nc.tensor.matmul(hps[:], wfs[:, k, :], xpr[:, k, :],
                                         start=(k == 0), stop=(k == 7))
                    hr = pf_sb.tile([128, OWN], bf16, tag="hr")
                    nc.scalar.activation(hr[:], hps[:], AF.Relu)
                    nc.vector.tensor_tensor(h2[:, hc, :], hr[:], hr[:], ALU.mult)

                mo_all = []
                for ohalf in range(2):
                    mo_ps = [pf_mo.tile([128, OWN], f32, name=f"mo{oi}",
                                        tag=f"mo{oi}") for oi in range(4)]
                    mo_all.append(mo_ps)

                for hc in range(34):
                    if hc < 32:
                        emit_fc(hc)
                    if hc >= 2:
                        hp = hc - 2
                        wps = pf_wp.tile([128, 4, 128], bf16, tag="wps")
                        nc.sync.dma_start(wps[:], wproj[hp, 0])
                        for oi in range(4):
                            nc.tensor.matmul(mo_all[0][oi][:], wps[:, oi, :],
                                             h2[:, hp, :],
                                             start=(hp == 0), stop=(hp == 31))
                # ohalf0 copies/squares run on scalar during the proj1 matmuls
                m2l = []
                for oi in range(4):
                    nc.scalar.activation(mout[:, oi, :], mo_all[0][oi][:],
                                         AF.Copy)
                    m2 = pf_sb.tile([128, OWN], bf16, tag="m2", bufs=8)
                    nc.scalar.activation(m2[:], mo_all[0][oi][:], AF.Square)
                    m2l.append(m2)
                for hc in range(32):
                    wps = pf_wp.tile([128, 4, 128], bf16, tag="wps")
                    nc.sync.dma_start(wps[:], wproj[hc, 1])
                    for oi in range(4):
                        nc.tensor.matmul(mo_all[1][oi][:], wps[:, oi, :],
                                         h2[:, hc, :],
                                         start=(hc == 0), stop=(hc == 31))
                    if hc < 4:
                        nc.tensor.matmul(ssm[:], onesc[:, 0:2], m2l[hc][:],
                                         start=(hc == 0), stop=False)
                for oi in range(4):
                    m2 = pf_sb.tile([128, OWN], bf16, tag="m2", bufs=8)
                    nc.scalar.activation(m2[:], mo_all[1][oi][:], AF.Square)
                    m2l.append(m2)
                for o in range(4, 8):
                    nc.tensor.matmul(ssm[:], onesc[:, 0:2], m2l[o][:],
                                     start=False, stop=(o == 7))
                invm = pf_sb.tile([2, OWN], INV_DT, tag="invm")
                if USE_ABSRSQRT:
                    nc.scalar.activation(invm[:], ssm[0:2, :],
                                         AF.Abs_reciprocal_sqrt,
                                         scale=1.0 / D, bias=eps6[0:2, :])
                else:
                    rmsm = pf_sb.tile([2, OWN], f32, tag="rmsm")
                    nc.scalar.activation(rmsm[:], ssm[0:2, :], AF.Sqrt,
                                         scale=1.0 / D, bias=eps6[0:2, :])
                    nc.vector.reciprocal_approx_fast(out=invm[:], in_=rmsm[:])
                repm = pf_sb.tile([128, OWN], INV_DT, tag="repm")
                nc.gpsimd.partition_broadcast(repm[:], invm[0:1, :], channels=128)
                for o in range(8):
                    msrc = mout[:, o, :] if o < 4 else mo_all[1][o - 4][:]
                    tmp = pf_sb.tile([128, OWN], f32, tag="tmpf")
                    nc.vector.scalar_tensor_tensor(
                        tmp[:], msrc, gml[:, o, None], repm[:],
                        ALU.mult, ALU.mult)
                    outv = pf_sb.tile([128, OWN], f32, tag="outv", bufs=4)
                    eng = nc.vector if o % 2 == 0 else nc.gpsimd
                    eng.tensor_tensor(outv[:], tmp[:], xpb[:, o, :], ALU.add)
                    nc.sync.dma_start(out_t[:, o, :], outv[:])

        rope_stack.close()

    nc.finalize()
    return nc


def _feat_major(a):
    """[F, T] -> device layout [128, F//128, T]."""
    F, T = a.shape
    return np.ascontiguousarray(a.reshape(F // 128, 128, T).transpose(1, 0, 2))


def _vec_dev(v):
    return np.ascontiguousarray(v.reshape(-1, 128).T)


def _bf(a):
    return np.ascontiguousarray(np.asarray(a, np.float32)).astype(ml_dtypes.bfloat16)


_CACHE = {}
_RUN_KW = {}



def kernel(x, attn_norm_w, mlp_norm_w, attn_post_norm_w, mlp_post_norm_w,
           attn_scale, mlp_scale, attn_mod_gain, attn_mod_bias,
           mlp_mod_gain, mlp_mod_bias, Wq, Wk, Wv, Wo, q_gain, fc_w, proj_w):
    x = np.asarray(x, np.float32)
    q_gain = np.asarray(q_gain, np.float32)

    if "nc" not in _CACHE:
        _CACHE["nc"] = build()
    nc = _CACHE["nc"]

    anw = np.asarray(attn_norm_w, np.float32)
    mnw = np.asarray(mlp_norm_w, np.float32)
    fc_eff = np.asarray(fc_w, np.float32) * mnw[None, :]

    # host-side input rmsnorm: xn = x * anw / rms1(x)
    ms1 = np.mean(np.square(x), axis=-1, keepdims=True)
    xn = (x * (1.0 / np.sqrt(ms1 + EPS_BLOCK))) * anw[None, None, :]

    # Wq columns permuted so p-tile p holds heads PAIRS[p] stacked (64+64)
    perm = np.zeros(D, np.int64)
    for p, (a, b) in enumerate(PAIRS):
        perm[p * 128:p * 128 + 64] = np.arange(a * 64, a * 64 + 64)
        perm[p * 128 + 64:(p + 1) * 128] = np.arange(b * 64, b * 64 + 64)
    WqTp = np.asarray(Wq, np.float32).T[:, perm]
    wq_dev = np.stack([_feat_major(WqTp[:, p * 128:(p + 1) * 128]) for p in range(8)])
    wk_dev = _feat_major(np.asarray(Wk, np.float32).T)
    wv_dev = _feat_major(np.asarray(Wv, np.float32).T)

    # Wo rows permuted to match y_all layout: chunk c2 = 4*half + 2*ab + i//2,
    # partition ph2*64+f  ->  original feature 64*(4*(2*half+ab) + i) + f,
    # with i = 2*(c2 % 2) + ph2.
    perm2 = np.zeros(D, np.int64)
    for c2 in range(8):
        halfg = c2 // 2          # kv-head index (2*half + ab)
        for ph2 in range(2):
            i = 2 * (c2 % 2) + ph2
            h_orig = 4 * halfg + i
            rows = np.arange(64)
            perm2[c2 * 128 + ph2 * 64 + rows] = 64 * h_orig + rows
    WoT = np.asarray(Wo, np.float32).T[perm2, :]
    wo_dev = np.stack([_feat_major(WoT[:, o * 128:(o + 1) * 128]) for o in range(8)])

    fcT = fc_eff.T
    wfc_dev = np.stack([_feat_major(fcT[:, h * 128:(h + 1) * 128]) for h in range(32)])
    projT = np.asarray(proj_w, np.float32).T                  # [4096, 1024]
    wproj_dev = np.ascontiguousarray(
        projT.reshape(32, 128, 2, 4, 128).transpose(0, 2, 1, 3, 4))

    # rope tables; sin sign-folded: x1-groups (even 32-blocks) get -sin
    inv_freq = 1.0 / (ROPE_BASE ** (np.arange(0, HD, 2, dtype=np.float32) / HD))
    tpos = np.arange(S, dtype=np.float32)
    freqs = np.outer(tpos, inv_freq).astype(np.float32)
    cosT = np.ascontiguousarray(np.tile(np.cos(freqs).T, (4, 1)))   # [128, S]
    sinN = np.ascontiguousarray(np.tile(np.sin(freqs).T, (4, 1)))
    # rope rotate-half matrix with signs: rot = R @ x (per 64-feature head)
    R = np.zeros((128, 128), np.float32)
    for p in range(128):
        if p % 64 < 32:
            R[p, p + 32] = 1.0
        else:
            R[p, p - 32] = -1.0
    rmat_h = np.ascontiguousarray(R.T)

    gat_v = (np.asarray(attn_post_norm_w, np.float32)
             * np.asarray(attn_mod_gain, np.float32)
             * np.asarray(attn_scale, np.float32))
    bat_v = np.asarray(attn_mod_bias, np.float32) * np.asarray(attn_scale, np.float32)
    gml_v = (np.asarray(mlp_post_norm_w, np.float32)
             * np.asarray(mlp_mod_gain, np.float32)
             * np.asarray(mlp_scale, np.float32))
    bml_v = np.asarray(mlp_mod_bias, np.float32) * np.asarray(mlp_scale, np.float32)

    gq_h = np.zeros((128, 8), np.float32)
    for p, (a, b) in enumerate(PAIRS):
        gq_h[0:64, p] = q_gain[a]
        gq_h[64:128, p] = q_gain[b]

    shared = {
        "wq": _bf(wq_dev), "wk": _bf(wk_dev), "wv": _bf(wv_dev),
        "wo": _bf(wo_dev), "wfc": _bf(wfc_dev), "wproj": _bf(wproj_dev),
        "cosF": _bf(cosT), "sinF": _bf(sinN), "rmat": _bf(rmat_h),
        "ones_c": _bf(np.ones((128, 2), np.float32)),
        "gq": gq_h,
        "g_attn": _vec_dev(gat_v), "g_mlp": _vec_dev(gml_v),
    }

    in_maps = []
    owners = []
    for c in range(8):
        b, j = c // 4, c % 4
        rows = np.concatenate(
            [np.arange((j + 4 * t) * 128, (j + 4 * t + 1) * 128) for t in range(4)])
        owners.append((b, rows))
        xnb = xn[b].T
        x_own_raw = x[b].T[:, rows]
        mask = np.zeros((4, 4, 128, 128), np.float32)
        for t in range(4):
            m = j + 4 * t
            q_idx = m * 128 + np.arange(128)
            for ktl in range(4):
                kv_idx = 512 * t + 128 * ktl + np.arange(128)
                mask[t, ktl] = (kv_idx[:, None] <= q_idx[None, :])
        m_in = {
            "xT": _bf(_feat_major(xnb)),
            "xq": _bf(_feat_major(xnb[:, rows])),
            "xres": _feat_major(x_own_raw + bat_v[:, None]),
            "xres2": _feat_major(x_own_raw + (bat_v + bml_v)[:, None]),
            "cosO": _bf(cosT[:, rows]),
            "sinO": _bf(sinN[:, rows]),
            "maskM": _bf(np.ascontiguousarray(mask.transpose(2, 0, 1, 3))),
        }
        m_in.update(shared)
        in_maps.append(m_in)

    res = run_bass_kernel_spmd(nc, in_maps, core_ids=list(range(8)),
                               **_RUN_KW)
    _CACHE["last_result"] = res

    out = np.empty((B, S, D), np.float32)
    for c in range(8):
        b, rows = owners[c]
        o = res.results[c]["out"]
        out[b, rows, :] = o.transpose(2, 1, 0).reshape(OWN, D)
    return out


# revision 23
# speedup vs baseline: 1.7951x; 1.0476x over previous
"""Trainium2 Bass kernel for one dense transformer block (B=2, S=2048, D=1024,
16 q-heads / 4 kv-heads GQA, squared-ReLU MLP), data-parallel over 8 NeuronCores.

Sharding: core c = (b, j), b = c // 4, j = c % 4, owns q-token tiles
{j, j+4, j+8, j+12} (128 tokens each) of batch b. K/V are computed for the full
sequence on every core (no collectives). The kv range for own q-tile t is
padded to 512*(t+1); causality enforced with per-core 0/1 masks on the
diagonal 512-wide kv chunk.

v2: bf16 matmul datapath (fp32 PSUM accumulation), host-side input rmsnorm
(xn = x*attn_norm_w/rms1 fed from DRAM), Q/K/V emission interleaved with
lagged norm + rope stages, rope rotate-half via a PE matmul with signs baked
into the R matrix, per-head normalizers replicated with gpsimd
partition_broadcast (q_gain folded into a per-partition stt scalar),
Abs_reciprocal_sqrt / reciprocal_approx_fast for all normalizers, exps
batched to 1024 elem/partition, AV matmuls lagged one tile behind QK+exp,
software-pipelined fc->proj MLP with the post-norm tail overlapped.

Numerical identities (exact up to negligible eps rescaling):
  - per-head q/k rmsnorm is scale-invariant per token -> Q/K project from the
    host-normalized xn without correction
  - the MLP input rmsnorm cancels through relu()^2 -> proj -> post-rmsnorm
  - no softmax max-subtraction (logits bounded by |q||k|/8 = 8)
  - softmax denominator = ones-columns appended to V in the AV matmul
"""

import os

import numpy as np
import ml_dtypes

import concourse.bass as bass
from concourse import bacc
import concourse.tile as tile
import concourse.mybir as mybir
from concourse.bass_utils import run_bass_kernel_spmd

f32 = mybir.dt.float32
f32r = mybir.dt.float32r
bf16 = mybir.dt.bfloat16
AF = mybir.ActivationFunctionType
ALU = mybir.AluOpType

B, S, D = 2, 2048, 1024
H, HKV, HD = 16, 4, 64
MLP_HID = 4 * D
KV = HKV * HD
NT = 16
OWN = 512
EPS_BLOCK = 1e-6
EPS_QK = float(np.finfo(np.float32).eps)
ROPE_BASE = 10000.0

PAIRS = [(0, 4), (1, 5), (2, 6), (3, 7), (8, 12), (9, 13), (10, 14), (11, 15)]

USE_ABSRSQRT = os.environ.get("KERNEL_ABSRSQRT", "1") == "1"
INV_DT = bf16 if USE_ABSRSQRT else f32

PHASE_ORDER = ["c", "ab", "d", "e", "f"]


def build():
    max_ph = os.environ.get("KERNEL_PHASES", "f")
    ph_on = lambda p: PHASE_ORDER.index(p) <= PHASE_ORDER.index(max_ph)
    bacc.Bacc.move_matmul_waits_to_ldweights = lambda self: None
    nc = bacc.Bacc(None)

    def dram_in(name, shape, dt=bf16):
        return nc.dram_tensor(name, list(shape), dt, kind="ExternalInput")

    xT = dram_in("xT", (128, 8, S))              # normalized x, feature-major
    xq = dram_in("xq", (128, 8, OWN))            # normalized own x
    xres = dram_in("xres", (128, 8, OWN), f32)   # raw own x + attn bias
    xres2 = dram_in("xres2", (128, 8, OWN), f32) # xres + mlp bias
    wq = dram_in("wq", (8, 128, 8, 128))
    wk = dram_in("wk", (128, 8, KV))
    wv = dram_in("wv", (128, 8, KV))
    wo = dram_in("wo", (8, 128, 8, 128))
    wfc = dram_in("wfc", (32, 128, 8, 128))
    wproj = dram_in("wproj", (32, 2, 128, 4, 128))
    cosF = dram_in("cosF", (128, S))
    sinF = dram_in("sinF", (128, S))
    cosO = dram_in("cosO", (128, OWN))
    sinO = dram_in("sinO", (128, OWN))
    rmat = dram_in("rmat", (128, 128))           # rope rotate-half (signs baked)
    maskM = dram_in("maskM", (128, 4, 4, 128))
    ones_c = dram_in("ones_c", (128, 2))
    gq = dram_in("gq", (128, 8), f32)            # per-p-tile head gains
    g_attn = dram_in("g_attn", (128, 8), f32)
    g_mlp = dram_in("g_mlp", (128, 8), f32)

    out_t = nc.dram_tensor("out", [128, 8, OWN], f32, kind="ExternalOutput")

    def rsqrt_into(pool, out_ap, in_ap, bias_ap, pfx):
        """out_ap = 1/sqrt(in_ap/HD + eps); in_ap [2, n] psum."""
        if USE_ABSRSQRT:
            nc.scalar.activation(out_ap, in_ap, AF.Abs_reciprocal_sqrt,
                                 scale=1.0 / HD, bias=bias_ap)
        else:
            rms = pool.tile([2, in_ap.shape[-1]], f32, name=f"{pfx}rms",
                            tag=f"{pfx}rms")
            nc.scalar.activation(rms[:], in_ap, AF.Sqrt, scale=1.0 / HD,
                                 bias=bias_ap)
            nc.vector.reciprocal_approx_fast(out=out_ap, in_=rms[:])

    with tile.TileContext(nc) as tc, \
         tc.tile_pool(name="cst", bufs=1) as cst, \
         tc.tile_pool(name="big", bufs=1) as big:
        # --- constants / tables -------------------------------------------
        onesc = cst.tile([128, 2], bf16, tag="onesc")
        nc.sync.dma_start(onesc[:], ones_c[:])
        gqt = cst.tile([128, 8], f32, tag="gqt")
        nc.sync.dma_start(gqt[:], gq[:])
        rmt = cst.tile([128, 128], bf16, tag="rmt")
        epsq = cst.tile([128, 1], f32, tag="epsq")
        nc.vector.memset(epsq[:], EPS_QK)
        eps6 = cst.tile([128, 1], f32, tag="eps6")
        nc.vector.memset(eps6[:], EPS_BLOCK)
        gat = cst.tile([128, 8], f32, tag="gat")
        gml = cst.tile([128, 8], f32, tag="gml")

        from contextlib import ExitStack
        rope_stack = ExitStack()
        ropep = rope_stack.enter_context(tc.tile_pool(name="ropep", bufs=1))
        coso = ropep.tile([128, OWN], bf16, tag="coso")
        sino = ropep.tile([128, OWN], bf16, tag="sino")
        cosf = ropep.tile([128, S], bf16, tag="cosf")
        sinf = ropep.tile([128, S], bf16, tag="sinf")

        kT = big.tile([128, 2, S], bf16, tag="kT")
        v_all = big.tile([128, 4, NT, 66], bf16, tag="v_all")
        qT = big.tile([128, 8, OWN], bf16, tag="qT_xpr")
        y_all = big.tile([128, 8, OWN], bf16, tag="yall")
        xrs = big.tile([128, 8, OWN], f32, tag="xrs")
        xrs2 = big.tile([128, 8, OWN], f32, tag="xrs2_mout")

        # ------------- Phases C+AB interleaved: Q, K, V -------------------
        if ph_on("c"):
            nc.vector.tensor_copy(
                v_all[:, :, :, 64:66],
                onesc[:, 0, None, None].to_broadcast([128, 4, NT, 2]))
            with tc.tile_pool(name="px", bufs=1) as px, \
                 tc.tile_pool(name="pxc", bufs=2) as pxc, \
                 tc.tile_pool(name="pw", bufs=1) as pw, \
                 tc.tile_pool(name="pwq", bufs=3) as pwq, \
                 tc.tile_pool(name="psb", bufs=3) as psb, \
                 tc.tile_pool(name="pqn", bufs=1) as pqn, \
                 tc.tile_pool(name="pps", bufs=1, space="PSUM") as pps, \
                 tc.tile_pool(name="pss", bufs=2, space="PSUM") as pss, \
                 tc.tile_pool(name="prt", bufs=2, space="PSUM") as prt:
                # critical-path DMAs first: first chunk + V/K weights + xq
                xcs = {}
                xc0 = pxc.tile([128, 8, 512], bf16, tag="xc")
                nc.sync.dma_start(xc0[:], xT[:, :, 0:512])
                xcs[0] = xc0
                wvs = pw.tile([128, 8, KV], bf16, tag="wvs")
                nc.sync.dma_start(wvs[:], wv[:])
                wks = pw.tile([128, 8, KV], bf16, tag="wks")
                nc.sync.dma_start(wks[:], wk[:])
                xqs = px.tile([128, 8, OWN], bf16, tag="xqs")
                nc.sync.dma_start(xqs[:], xq[:])
                # long-lead tables and residuals after the critical batch
                nc.sync.dma_start(rmt[:], rmat[:])
                nc.sync.dma_start(coso[:], cosO[:])
                nc.sync.dma_start(sino[:], sinO[:])
                nc.sync.dma_start(cosf[:], cosF[:])
                nc.sync.dma_start(sinf[:], sinF[:])
                nc.sync.dma_start(xrs[:], xres[:])
                nc.sync.dma_start(xrs2[:], xres2[:])
                nc.sync.dma_start(gat[:], g_attn[:])
                nc.sync.dma_start(gml[:], g_mlp[:])

                def unit_v(ci):
                    # V token-major for the 4 token tiles of chunk ci
                    xc = xcs[ci]
                    for kt in range(4):
                        gkt = ci * 4 + kt
                        vps = pps.tile([128, KV], f32, tag="vps", bufs=1)
                        for k in range(8):
                            nc.tensor.matmul(vps[:], xc[:, k, kt * 128:(kt + 1) * 128],
                                             wvs[:, k, :], start=(k == 0), stop=(k == 7))
                        nc.scalar.activation(
                            v_all[:, :, gkt, 0:64],
                            vps[:].rearrange("p (g d) -> p g d", g=4), AF.Copy)

                def unit_proj(u):
                    kind = u[0]
                    if kind == "Q":
                        p = u[1]
                        wqs = pwq.tile([128, 8, 128], bf16, tag="wqs")
                        nc.sync.dma_start(wqs[:], wq[p])
                        ps = pps.tile([128, OWN], f32, tag="pps", bufs=3)
                        for k in range(8):
                            nc.tensor.matmul(ps[:], wqs[:, k, :], xqs[:, k, :],
                                             start=(k == 0), stop=(k == 7))
                    else:
                        ci, kp = u[1], u[2]
                        xc = xcs[ci]
                        ps = pps.tile([128, OWN], f32, tag="pps", bufs=3)
                        for k in range(8):
                            nc.tensor.matmul(ps[:], wks[:, k, kp * 128:(kp + 1) * 128],
                                             xc[:, k, :], start=(k == 0), stop=(k == 7))
                    sq = psb.tile([128, OWN], bf16, tag="sq")
                    nc.scalar.activation(sq[:], ps[:], AF.Square)
                    return (u, ps, sq)

                def unit_norm(st):
                    u, ps, sq = st
                    inv = psb.tile([2, 2, OWN], INV_DT, tag="inv")
                    for half in range(2):
                        hs = slice(half * 64, (half + 1) * 64)
                        ss = pss.tile([2, OWN], f32, tag="ss")
                        nc.tensor.matmul(ss[:], onesc[hs, 0:2], sq[hs, :],
                                         start=True, stop=True,
                                         tile_position=(half * 64, 0))
                        rsqrt_into(psb, inv[0:2, half, :], ss[0:2, :],
                                   epsq[0:2, :], "n")
                    rep = psb.tile([128, 2, OWN], INV_DT, tag="rep")
                    nc.gpsimd.partition_broadcast(rep[:], inv[0:1, :, :],
                                                  channels=128)
                    pbf = psb.tile([128, OWN], bf16, tag="pbf")
                    nc.scalar.activation(pbf[:], ps[:], AF.Copy)
                    if u[0] == "Q":
                        xn = pqn.tile([128, OWN], bf16, tag="qn", bufs=8)
                        for half in range(2):
                            hs = slice(half * 64, (half + 1) * 64)
                            nc.vector.scalar_tensor_tensor(
                                xn[hs, :], pbf[hs, :], gqt[hs, u[1], None],
                                rep[hs, half, :], ALU.mult, ALU.mult)
                    else:
                        xn = pqn.tile([128, OWN], bf16, tag="kn", bufs=4)
                        for half in range(2):
                            hs = slice(half * 64, (half + 1) * 64)
                            nc.vector.tensor_tensor(xn[hs, :], pbf[hs, :],
                                                    rep[hs, half, :], ALU.mult)
                    return (u, xn)

                def unit_rope(st):
                    u, xn = st
                    rot = prt.tile([128, OWN], f32, tag="rot")
                    nc.tensor.matmul(rot[:], rmt[:], xn[:], start=True, stop=True)
                    t1 = psb.tile([128, OWN], bf16, tag="t1")
                    t2 = psb.tile([128, OWN], bf16, tag="t2")
                    if u[0] == "Q":
                        nc.vector.tensor_tensor(t1[:], xn[:], coso[:], ALU.mult)
                        nc.vector.tensor_tensor(t2[:], rot[:], sino[:], ALU.mult)
                        nc.vector.tensor_tensor(qT[:, u[1], :], t1[:], t2[:],
                                                ALU.add)
                    else:
                        ci, kp = u[1], u[2]
                        sl = slice(ci * 512, (ci + 1) * 512)
                        nc.vector.tensor_tensor(t1[:], xn[:], cosf[:, sl], ALU.mult)
                        nc.vector.tensor_tensor(t2[:], rot[:], sinf[:, sl], ALU.mult)
                        nc.vector.tensor_tensor(kT[:, kp, sl], t1[:], t2[:],
                                                ALU.add)

                units = []
                qp = 0
                for ci in range(4):
                    units.append(("V", ci))
                    units.append(("K", ci, 0))
                    units.append(("K", ci, 1))
                    units.append(("Q", qp)); qp += 1
                    units.append(("Q", qp)); qp += 1

                normq, ropeq = [], []
                for u in units:
                    if u[0] == "V":
                        ci = u[1]
                        if ci + 1 < 4:
                            xcn = pxc.tile([128, 8, 512], bf16, tag="xc")
                            nc.sync.dma_start(xcn[:],
                                              xT[:, :, (ci + 1) * 512:(ci + 2) * 512])
                            xcs[ci + 1] = xcn
                        unit_v(ci)
                        continue
                    normq.append(unit_proj(u))
                    if len(normq) >= 2:
                        ropeq.append(unit_norm(normq.pop(0)))
                    if len(ropeq) >= 3:
                        unit_rope(ropeq.pop(0))
                while normq:
                    ropeq.append(unit_norm(normq.pop(0)))
                while ropeq:
                    unit_rope(ropeq.pop(0))

        # ------------- Phase D: attention ---------------------------------
        wo_stack = ExitStack()
        pe_w = wo_stack.enter_context(tc.tile_pool(name="pe_w", bufs=1))
        wosl = []
        for o in range(8):
            wos = pe_w.tile([128, 8, 128], bf16, tag=f"wos{o}")
            wosl.append(wos)
        if ph_on("d"):
            for o in range(8):
                nc.sync.dma_start(wosl[o][:], wo[o])
            with tc.tile_pool(name="pd_m", bufs=1) as pd_m, \
                 tc.tile_pool(name="pd_pt", bufs=6) as pd_pt, \
                 tc.tile_pool(name="pd_sb", bufs=3) as pd_sb, \
                 tc.tile_pool(name="pd_s", bufs=2, space="PSUM") as pd_s, \
                 tc.tile_pool(name="pd_y", bufs=2, space="PSUM") as pd_y:
                masks = pd_m.tile([128, 4, 4, 128], bf16, tag="masks")
                nc.sync.dma_start(masks[:], maskM[:])
                for t in range(4):
                    qsl = slice(t * 128, (t + 1) * 128)
                    n_chunks = t + 1
                    n_kvt = 4 * n_chunks
                    for half in range(2):
                        gA, gB = 2 * half, 2 * half + 1
                        yA = pd_y.tile([66, 4, 128], f32, tag="yA")
                        yB = pd_y.tile([66, 4, 128], f32, tag="yB")
                        qsA = qT[0:64, 4 * half:4 * half + 4, qsl]
                        qsB = qT[64:128, 4 * half:4 * half + 4, qsl]
                        av_pend = []

                        def emit_av(c, i2, ptA, ptB):
                            for isub in range(2):
                                kvt = 4 * c + 2 * i2 + isub
                                nc.tensor.matmul(yA[:], v_all[:, gA, kvt, :],
                                                 ptA[:, isub, :, :],
                                                 start=(kvt == 0),
                                                 stop=(kvt == n_kvt - 1))
                                nc.tensor.matmul(yB[:], v_all[:, gB, kvt, :],
                                                 ptB[:, isub, :, :],
                                                 start=(kvt == 0),
                                                 stop=(kvt == n_kvt - 1))

                        for c in range(n_chunks):
                            for i2 in range(2):
                                sA = pd_s.tile([128, 2, 4, 128], f32, tag="s")
                                sB = pd_s.tile([128, 2, 4, 128], f32, tag="s")
                                for isub in range(2):
                                    i = 2 * i2 + isub
                                    ks = slice((4 * c + i) * 128,
                                               (4 * c + i + 1) * 128)
                                    nc.tensor.matmul(sA[:, isub, :, :],
                                                     kT[0:64, half, ks], qsA,
                                                     start=True, stop=True,
                                                     tile_position=(0, 0))
                                    nc.tensor.matmul(sB[:, isub, :, :],
                                                     kT[64:128, half, ks], qsB,
                                                     start=True, stop=True,
                                                     tile_position=(64, 0))
                                ptA = pd_pt.tile([128, 2, 4, 128], bf16, tag="ptA")
                                ptB = pd_pt.tile([128, 2, 4, 128], bf16, tag="ptB")
                                nc.scalar.activation(ptA[:], sA[:], AF.Exp,
                                                     scale=0.125)
                                nc.scalar.activation(ptB[:], sB[:], AF.Exp,
                                                     scale=0.125)
                                if c == t:
                                    mbc = masks[:, t, 2 * i2:2 * i2 + 2, None, :] \
                                        .to_broadcast([128, 2, 4, 128])
                                    nc.vector.tensor_tensor(ptA[:], ptA[:], mbc,
                                                            ALU.mult)
                                    nc.vector.tensor_tensor(ptB[:], ptB[:], mbc,
                                                            ALU.mult)
                                av_pend.append((c, i2, ptA, ptB))
                                if len(av_pend) > 1:
                                    emit_av(*av_pend.pop(0))
                        while av_pend:
                            emit_av(*av_pend.pop(0))
                        for ab, y in ((0, yA), (1, yB)):
                            dn = pd_sb.tile([2, 4, 128], f32, tag="dn")
                            nc.vector.tensor_copy(dn[0:2, :, :], y[64:66, :, :])
                            invy = pd_sb.tile([2, 4, 128], f32, tag="invy")
                            nc.vector.reciprocal_approx_fast(
                                out=invy[:], in_=dn[:])
                            repy = pd_sb.tile([128, 4, 128], f32, tag="repy")
                            nc.gpsimd.partition_broadcast(
                                repy[:], invy[0:1, :, :], channels=128)
                            c20 = 4 * half + 2 * ab
                            y4 = y.rearrange("p (a b) q -> p a b q", b=2)
                            r4 = repy.rearrange("p (a b) q -> p a b q", b=2)
                            for ph2 in range(2):
                                nc.vector.tensor_tensor(
                                    y_all[ph2 * 64:ph2 * 64 + 64,
                                          c20:c20 + 2, qsl],
                                    y4[0:64, :, ph2, :],
                                    r4[0:64, :, ph2, :], ALU.mult)

        # ------------- Phase E: Wo + post-norm + residual -----------------
        if ph_on("e"):
            xpr = big.tile([128, 8, OWN], bf16, tag="qT_xpr")
            xpb = big.tile([128, 8, OWN], f32, tag="xpb")
            with tc.tile_pool(name="pe_sb", bufs=2) as pe_sb, \
                 tc.tile_pool(name="pe_ao", bufs=1) as pe_ao, \
                 tc.tile_pool(name="pe_ps", bufs=3, space="PSUM") as pe_ps, \
                 tc.tile_pool(name="pe_ps1", bufs=1, space="PSUM") as pe_ps1:
                ao = pe_ao.tile([128, 8, OWN], f32, tag="ao")
                ssa = pe_ps1.tile([2, OWN], f32, tag="ssa")
                a2l = []
                for o in range(8):
                    wos = wosl[o]
                    aps = pe_ps.tile([128, OWN], f32, tag="aps")
                    for k in range(8):
                        nc.tensor.matmul(aps[:], wos[:, k, :], y_all[:, k, :],
                                         start=(k == 0), stop=(k == 7))
                    nc.scalar.activation(ao[:, o, :], aps[:], AF.Copy)
                    a2 = pe_sb.tile([128, OWN], bf16, tag="a2", bufs=3)
                    nc.scalar.activation(a2[:], aps[:], AF.Square)
                    a2l.append(a2)
                    if o >= 1:
                        nc.tensor.matmul(ssa[:], onesc[:, 0:2], a2l[o - 1][:],
                                         start=(o == 1), stop=False)
                nc.tensor.matmul(ssa[:], onesc[:, 0:2], a2l[7][:],
                                 start=False, stop=True)
                inva = pe_sb.tile([2, OWN], INV_DT, tag="inva")
                if USE_ABSRSQRT:
                    nc.scalar.activation(inva[:], ssa[0:2, :],
                                         AF.Abs_reciprocal_sqrt,
                                         scale=1.0 / D, bias=eps6[0:2, :])
                else:
                    rmsa = pe_sb.tile([2, OWN], f32, tag="rmsa")
                    nc.scalar.activation(rmsa[:], ssa[0:2, :], AF.Sqrt,
                                         scale=1.0 / D, bias=eps6[0:2, :])
                    nc.vector.reciprocal_approx_fast(out=inva[:], in_=rmsa[:])
                repa = pe_sb.tile([128, OWN], INV_DT, tag="repa")
                nc.gpsimd.partition_broadcast(repa[:], inva[0:1, :], channels=128)
                tmps = []
                for o in range(8):
                    tmp = pe_sb.tile([128, OWN], f32, tag="tmpe", bufs=8)
                    nc.vector.scalar_tensor_tensor(
                        tmp[:], ao[:, o, :], gat[:, o, None], repa[:],
                        ALU.mult, ALU.mult)
                    nc.vector.tensor_tensor(xpr[:, o, :], tmp[:], xrs[:, o, :],
                                            ALU.add)
                    tmps.append(tmp)
                for o in range(8):
                    nc.gpsimd.tensor_tensor(xpb[:, o, :], tmps[o][:],
                                            xrs2[:, o, :], ALU.add)
            wo_stack.close()

        # ------------- Phase F: MLP ---------------------------------------
        if ph_on("f"):
            mout = big.tile([128, 8, OWN], f32, tag="xrs2_mout")
            with tc.tile_pool(name="pf_h2", bufs=1) as pf_h2, \
                 tc.tile_pool(name="pf_sb", bufs=3) as pf_sb, \
                 tc.tile_pool(name="pf_wf", bufs=3) as pf_wf, \
                 tc.tile_pool(name="pf_wp", bufs=3) as pf_wp, \
                 tc.tile_pool(name="pf_ps", bufs=2, space="PSUM") as pf_ps, \
                 tc.tile_pool(name="pf_mo", bufs=1, space="PSUM") as pf_mo, \
                 tc.tile_pool(name="pf_ss", bufs=1, space="PSUM") as pf_ss:
                h2 = pf_h2.tile([128, 32, OWN], bf16, tag="h2")
                ssm = pf_ss.tile([2, OWN], f32, tag="ssm")

                def emit_fc(hc):
                    wfs = pf_wf.tile([128, 8, 128], bf16, tag="wfs")
                    nc.sync.dma_start(wfs[:], wfc[hc])
                    hps = pf_ps.tile([128, OWN], f32, tag="hps")
                    for k in range(8):
                        nc.tensor.matmul(hps[:], wfs[:, k, :], xpr[:, k, :],
                                         start=(k == 0), stop=(k == 7))
                    hr = pf_sb.tile([128, OWN], bf16, tag="hr")
                    nc.scalar.activation(hr[:], hps[:], AF.Relu)
                    nc.vector.tensor_tensor(h2[:, hc, :], hr[:], hr[:], ALU.mult)

                mo_all = []
                for ohalf in range(2):
                    mo_ps = [pf_mo.tile([128, OWN], f32, name=f"mo{oi}",
                                        tag=f"mo{oi}") for oi in range(4)]
                    mo_all.append(mo_ps)

                for hc in range(34):
                    if hc < 32:
                        emit_fc(hc)
                    if hc >= 2:
                        hp = hc - 2
                        wps = pf_wp.tile([128, 4, 128], bf16, tag="wps")
                        nc.sync.dma_start(wps[:], wproj[hp, 0])
                        for oi in range(4):
                            nc.tensor.matmul(mo_all[0][oi][:], wps[:, oi, :],
                                             h2[:, hp, :],
                                             start=(hp == 0), stop=(hp == 31))
                # ohalf0 copies/squares run on scalar during the proj1 matmuls
                m2l = []
                for oi in range(4):
                    nc.scalar.activation(mout[:, oi, :], mo_all[0][oi][:],
                                         AF.Copy)
                    m2 = pf_sb.tile([128, OWN], bf16, tag="m2", bufs=8)
                    nc.scalar.activation(m2[:], mo_all[0][oi][:], AF.Square)
                    m2l.append(m2)
                for hc in range(32):
                    wps = pf_wp.tile([128, 4, 128], bf16, tag="wps")
                    nc.sync.dma_start(wps[:], wproj[hc, 1])
                    for oi in range(4):
                        nc.tensor.matmul(mo_all[1][oi][:], wps[:, oi, :],
                                         h2[:, hc, :],
                                         start=(hc == 0), stop=(hc == 31))
                    if hc < 4:
                        nc.tensor.matmul(ssm[:], onesc[:, 0:2], m2l[hc][:],
                                         start=(hc == 0), stop=False)
                for oi in range(4):
                    m2 = pf_sb.tile([128, OWN], bf16, tag="m2", bufs=8)
                    nc.scalar.activation(m2[:], mo_all[1][oi][:], AF.Square)
                    m2l.append(m2)
                for o in range(4, 8):
                    nc.tensor.matmul(ssm[:], onesc[:, 0:2], m2l[o][:],
                                     start=False, stop=(o == 7))
                invm = pf_sb.tile([2, OWN], INV_DT, tag="invm")
                if USE_ABSRSQRT:
                    nc.scalar.activation(invm[:], ssm[0:2, :],
                                         AF.Abs_reciprocal_sqrt,
                                         scale=1.0 / D, bias=eps6[0:2, :])
                else:
                    rmsm = pf_sb.tile([2, OWN], f32, tag="rmsm")
                    nc.scalar.activation(rmsm[:], ssm[0:2, :], AF.Sqrt,
                                         scale=1.0 / D, bias=eps6[0:2, :])
                    nc.vector.reciprocal_approx_fast(out=invm[:], in_=rmsm[:])
                repm = pf_sb.tile([128, OWN], INV_DT, tag="repm")
                nc.gpsimd.partition_broadcast(repm[:], invm[0:1, :], channels=128)
                for o in range(8):
                    msrc = mout[:, o, :] if o < 4 else mo_all[1][o - 4][:]
                    tmp = pf_sb.tile([128, OWN], f32, tag="tmpf")
                    nc.vector.scalar_tensor_tensor(
                        tmp[:], msrc, gml[:, o, None], repm[:],
                        ALU.mult, ALU.mult)
                    outv = pf_sb.tile([128, OWN], f32, tag="outv", bufs=4)
                    eng = nc.vector if o % 2 == 0 else nc.gpsimd
                    eng.tensor_tensor(outv[:], tmp[:], xpb[:, o, :], ALU.add)
                    nc.sync.dma_start(out_t[:, o, :], outv[:])

        rope_stack.close()

    nc.finalize()
    return nc


def _feat_major(a):
    """[F, T] -> device layout [128, F//128, T]."""
    F, T = a.shape
    return np.ascontiguousarray(a.reshape(F // 128, 128, T).transpose(1, 0, 2))


def _vec_dev(v):
    return np.ascontiguousarray(v.reshape(-1, 128).T)


def _bf(a):
    return np.ascontiguousarray(np.asarray(a, np.float32)).astype(ml_dtypes.bfloat16)


_CACHE = {}
_RUN_KW = {}



def kernel(x, attn_norm_w, mlp_norm_w, attn_post_norm_w, mlp_post_norm_w,
           attn_scale, mlp_scale, attn_mod_gain, attn_mod_bias,
           mlp_mod_gain, mlp_mod_bias, Wq, Wk, Wv, Wo, q_gain, fc_w, proj_w):
    x = np.asarray(x, np.float32)
    q_gain = np.asarray(q_gain, np.float32)

    if "nc" not in _CACHE:
        _CACHE["nc"] = build()
    nc = _CACHE["nc"]

    anw = np.asarray(attn_norm_w, np.float32)
    mnw = np.asarray(mlp_norm_w, np.float32)
    fc_eff = np.asarray(fc_w, np.float32) * mnw[None, :]

    # host-side input rmsnorm: xn = x * anw / rms1(x)
    ms1 = np.mean(np.square(x), axis=-1, keepdims=True)
    xn = (x * (1.0 / np.sqrt(ms1 + EPS_BLOCK))) * anw[None, None, :]

    # Wq columns permuted so p-tile p holds heads PAIRS[p] stacked (64+64)
    perm = np.zeros(D, np.int64)
    for p, (a, b) in enumerate(PAIRS):
        perm[p * 128:p * 128 + 64] = np.arange(a * 64, a * 64 + 64)
        perm[p * 128 + 64:(p + 1) * 128] = np.arange(b * 64, b * 64 + 64)
    WqTp = np.asarray(Wq, np.float32).T[:, perm]
    wq_dev = np.stack([_feat_major(WqTp[:, p * 128:(p + 1) * 128]) for p in range(8)])
    wk_dev = _feat_major(np.asarray(Wk, np.float32).T)
    wv_dev = _feat_major(np.asarray(Wv, np.float32).T)

    # Wo rows permuted to match y_all layout: chunk c2 = 4*half + 2*ab + i//2,
    # partition ph2*64+f  ->  original feature 64*(4*(2*half+ab) + i) + f,
    # with i = 2*(c2 % 2) + ph2.
    perm2 = np.zeros(D, np.int64)
    for c2 in range(8):
        halfg = c2 // 2          # kv-head index (2*half + ab)
        for ph2 in range(2):
            i = 2 * (c2 % 2) + ph2
            h_orig = 4 * halfg + i
            rows = np.arange(64)
            perm2[c2 * 128 + ph2 * 64 + rows] = 64 * h_orig + rows
    WoT = np.asarray(Wo, np.float32).T[perm2, :]
    wo_dev = np.stack([_feat_major(WoT[:, o * 128:(o + 1) * 128]) for o in range(8)])

    fcT = fc_eff.T
    wfc_dev = np.stack([_feat_major(fcT[:, h * 128:(h + 1) * 128]) for h in range(32)])
    projT = np.asarray(proj_w, np.float32).T                  # [4096, 1024]
    wproj_dev = np.ascontiguousarray(
        projT.reshape(32, 128, 2, 4, 128).transpose(0, 2, 1, 3, 4))

    # rope tables; sin sign-folded: x1-groups (even 32-blocks) get -sin
    inv_freq = 1.0 / (ROPE_BASE ** (np.arange(0, HD, 2, dtype=np.float32) / HD))
    tpos = np.arange(S, dtype=np.float32)
    freqs = np.outer(tpos, inv_freq).astype(np.float32)
    cosT = np.ascontiguousarray(np.tile(np.cos(freqs).T, (4, 1)))   # [128, S]
    sinN = np.ascontiguousarray(np.tile(np.sin(freqs).T, (4, 1)))
    # rope rotate-half matrix with signs: rot = R @ x (per 64-feature head)
    R = np.zeros((128, 128), np.float32)
    for p in range(128):
        if p % 64 < 32:
            R[p, p + 32] = 1.0
        else:
            R[p, p - 32] = -1.0
    rmat_h = np.ascontiguousarray(R.T)

    gat_v = (np.asarray(attn_post_norm_w, np.float32)
             * np.asarray(attn_mod_gain, np.float32)
             * np.asarray(attn_scale, np.float32))
    bat_v = np.asarray(attn_mod_bias, np.float32) * np.asarray(attn_scale, np.float32)
    gml_v = (np.asarray(mlp_post_norm_w, np.float32)
             * np.asarray(mlp_mod_gain, np.float32)
             * np.asarray(mlp_scale, np.float32))
    bml_v = np.asarray(mlp_mod_bias, np.float32) * np.asarray(mlp_scale, np.float32)

    gq_h = np.zeros((128, 8), np.float32)
    for p, (a, b) in enumerate(PAIRS):
        gq_h[0:64, p] = q_gain[a]
        gq_h[64:128, p] = q_gain[b]

    shared = {
        "wq": _bf(wq_dev), "wk": _bf(wk_dev), "wv": _bf(wv_dev),
        "wo": _bf(wo_dev), "wfc": _bf(wfc_dev), "wproj": _bf(wproj_dev),
        "cosF": _bf(cosT), "sinF": _bf(sinN), "rmat": _bf(rmat_h),
        "ones_c": _bf(np.ones((128, 2), np.float32)),
        "gq": gq_h,
        "g_attn": _vec_dev(gat_v), "g_mlp": _vec_dev(gml_v),
    }

    in_maps = []
    owners = []
    for c in range(8):
        b, j = c // 4, c % 4
        rows = np.concatenate(
            [np.arange((j + 4 * t) * 128, (j + 4 * t + 1) * 128) for t in range(4)])
        owners.append((b, rows))
        xnb = xn[b].T
        x_own_raw = x[b].T[:, rows]
        mask = np.zeros((4, 4, 128, 128), np.float32)
        for t in range(4):
            m = j + 4 * t
            q_idx = m * 128 + np.arange(128)
            for ktl in range(4):
                kv_idx = 512 * t + 128 * ktl + np.arange(128)
                mask[t, ktl] = (kv_idx[:, None] <= q_idx[None, :])
        m_in = {
            "xT": _bf(_feat_major(xnb)),
            "xq": _bf(_feat_major(xnb[:, rows])),
            "xres": _feat_major(x_own_raw + bat_v[:, None]),
            "xres2": _feat_major(x_own_raw + (bat_v + bml_v)[:, None]),
            "cosO": _bf(cosT[:, rows]),
            "sinO": _bf(sinN[:, rows]),
            "maskM": _bf(np.ascontiguousarray(mask.transpose(2, 0, 1, 3))),
        }
        m_in.update(shared)
        in_maps.append(m_in)

    res = run_bass_kernel_spmd(nc, in_maps, core_ids=list(range(8)),
                               **_RUN_KW)
    _CACHE["last_result"] = res

    out = np.empty((B, S, D), np.float32)
    for c in range(8):
        b, rows = owners[c]
        o = res.results[c]["out"]
        out[b, rows, :] = o.transpose(2, 1, 0).reshape(OWN, D)
    return out


# revision 26
# speedup vs baseline: 1.8671x; 1.0401x over previous
"""Trainium2 Bass kernel for one dense transformer block (B=2, S=2048, D=1024,
16 q-heads / 4 kv-heads GQA, squared-ReLU MLP), data-parallel over 8 NeuronCores.

Sharding: core c = (b, j), b = c // 4, j = c % 4, owns q-token tiles
{j, j+4, j+8, j+12} (128 tokens each) of batch b. K/V are computed for the full
sequence on every core (no collectives). The kv range for own q-tile t is
padded to 512*(t+1); causality enforced with per-core 0/1 masks on the
diagonal 512-wide kv chunk.

v2: bf16 matmul datapath (fp32 PSUM accumulation), host-side input rmsnorm
(xn = x*attn_norm_w/rms1 fed from DRAM), Q/K/V emission interleaved with
lagged norm + rope stages, rope rotate-half via a PE matmul with signs baked
into the R matrix, per-head normalizers replicated with gpsimd
partition_broadcast (q_gain folded into a per-partition stt scalar),
Abs_reciprocal_sqrt / reciprocal_approx_fast for all normalizers, exps
batched to 1024 elem/partition, AV matmuls lagged one tile behind QK+exp,
software-pipelined fc->proj MLP with the post-norm tail overlapped.

Numerical identities (exact up to negligible eps rescaling):
  - per-head q/k rmsnorm is scale-invariant per token -> Q/K project from the
    host-normalized xn without correction
  - the MLP input rmsnorm cancels through relu()^2 -> proj -> post-rmsnorm
  - no softmax max-subtraction (logits bounded by |q||k|/8 = 8)
  - softmax denominator = ones-columns appended to V in the AV matmul
"""

import os

import numpy as np
import ml_dtypes

import concourse.bass as bass
from concourse import bacc
import concourse.tile as tile
import concourse.mybir as mybir
from concourse.bass_utils import run_bass_kernel_spmd

f32 = mybir.dt.float32
f32r = mybir.dt.float32r
bf16 = mybir.dt.bfloat16
AF = mybir.ActivationFunctionType
ALU = mybir.AluOpType

B, S, D = 2, 2048, 1024
H, HKV, HD = 16, 4, 64
MLP_HID = 4 * D
KV = HKV * HD
NT = 16
OWN = 512
EPS_BLOCK = 1e-6
EPS_QK = float(np.finfo(np.float32).eps)
ROPE_BASE = 10000.0

PAIRS = [(0, 4), (1, 5), (2, 6), (3, 7), (8, 12), (9, 13), (10, 14), (11, 15)]

USE_ABSRSQRT = os.environ.get("KERNEL_ABSRSQRT", "1") == "1"
INV_DT = bf16 if USE_ABSRSQRT else f32

PHASE_ORDER = ["c", "ab", "d", "e", "f"]


def build():
    max_ph = os.environ.get("KERNEL_PHASES", "f")
    ph_on = lambda p: PHASE_ORDER.index(p) <= PHASE_ORDER.index(max_ph)
    bacc.Bacc.move_matmul_waits_to_ldweights = lambda self: None
    nc = bacc.Bacc(None)

    def dram_in(name, shape, dt=bf16):
        return nc.dram_tensor(name, list(shape), dt, kind="ExternalInput")

    xT = dram_in("xT", (128, 8, S))              # normalized x, feature-major
    xq = dram_in("xq", (128, 8, OWN))            # normalized own x
    xres = dram_in("xres", (128, 8, OWN), f32)   # raw own x + attn bias
    xres2 = dram_in("xres2", (128, 8, OWN), f32) # xres + mlp bias
    wq = dram_in("wq", (8, 128, 8, 128))
    wk = dram_in("wk", (128, 8, KV))
    wv = dram_in("wv", (128, 8, KV))
    wo = dram_in("wo", (8, 128, 8, 128))
    wfc = dram_in("wfc", (32, 128, 8, 128))
    wproj = dram_in("wproj", (32, 2, 128, 4, 128))
    cosF = dram_in("cosF", (128, S))
    sinF = dram_in("sinF", (128, S))
    cosO = dram_in("cosO", (128, OWN))
    sinO = dram_in("sinO", (128, OWN))
    rmat = dram_in("rmat", (128, 128))           # rope rotate-half (signs baked)
    maskM = dram_in("maskM", (128, 4, 4, 128))
    ones_c = dram_in("ones_c", (128, 2))
    gq = dram_in("gq", (128, 8), f32)            # per-p-tile head gains
    g_attn = dram_in("g_attn", (128, 8), f32)
    g_mlp = dram_in("g_mlp", (128, 8), f32)

    out_t = nc.dram_tensor("out", [128, 8, OWN], f32, kind="ExternalOutput")

    def rsqrt_into(pool, out_ap, in_ap, bias_ap, pfx):
        """out_ap = 1/sqrt(in_ap/HD + eps); in_ap [2, n] psum."""
        if USE_ABSRSQRT:
            nc.scalar.activation(out_ap, in_ap, AF.Abs_reciprocal_sqrt,
                                 scale=1.0 / HD, bias=bias_ap)
        else:
            rms = pool.tile([2, in_ap.shape[-1]], f32, name=f"{pfx}rms",
                            tag=f"{pfx}rms")
            nc.scalar.activation(rms[:], in_ap, AF.Sqrt, scale=1.0 / HD,
                                 bias=bias_ap)
            nc.vector.reciprocal_approx_fast(out=out_ap, in_=rms[:])

    with tile.TileContext(nc) as tc, \
         tc.tile_pool(name="cst", bufs=1) as cst, \
         tc.tile_pool(name="big", bufs=1) as big:
        # --- constants / tables -------------------------------------------
        onesc = cst.tile([128, 2], bf16, tag="onesc")
        nc.sync.dma_start(onesc[:], ones_c[:])
        gqt = cst.tile([128, 8], f32, tag="gqt")
        nc.sync.dma_start(gqt[:], gq[:])
        rmt = cst.tile([128, 128], bf16, tag="rmt")
        epsq = cst.tile([128, 1], f32, tag="epsq")
        nc.vector.memset(epsq[:], EPS_QK)
        eps6 = cst.tile([128, 1], f32, tag="eps6")
        nc.vector.memset(eps6[:], EPS_BLOCK)
        gat = cst.tile([128, 8], f32, tag="gat")
        gml = cst.tile([128, 8], f32, tag="gml")

        from contextlib import ExitStack
        rope_stack = ExitStack()
        ropep = rope_stack.enter_context(tc.tile_pool(name="ropep", bufs=1))
        coso = ropep.tile([128, OWN], bf16, tag="coso")
        sino = ropep.tile([128, OWN], bf16, tag="sino")
        cosf = ropep.tile([128, S], bf16, tag="cosf")
        sinf = ropep.tile([128, S], bf16, tag="sinf")

        kT = big.tile([128, 2, S], bf16, tag="kT")
        v_all = big.tile([128, 4, NT, 66], bf16, tag="v_all")
        qT = big.tile([128, 8, OWN], bf16, tag="qT_xpr")
        y_all = big.tile([128, 8, OWN], bf16, tag="yall")
        xrs = big.tile([128, 8, OWN], f32, tag="xrs")
        xrs2 = big.tile([128, 8, OWN], f32, tag="xrs2_mout")

        # ------------- Phases C+AB interleaved: Q, K, V -------------------
        if ph_on("c"):
            nc.vector.tensor_copy(
                v_all[:, :, :, 64:66],
                onesc[:, 0, None, None].to_broadcast([128, 4, NT, 2]))
            with tc.tile_pool(name="px", bufs=1) as px, \
                 tc.tile_pool(name="pxc", bufs=2) as pxc, \
                 tc.tile_pool(name="pw", bufs=1) as pw, \
                 tc.tile_pool(name="pwq", bufs=3) as pwq, \
                 tc.tile_pool(name="psb", bufs=3) as psb, \
                 tc.tile_pool(name="pqn", bufs=1) as pqn, \
                 tc.tile_pool(name="pps", bufs=1, space="PSUM") as pps, \
                 tc.tile_pool(name="pss", bufs=2, space="PSUM") as pss, \
                 tc.tile_pool(name="prt", bufs=2, space="PSUM") as prt:
                # critical-path DMAs first: first chunk + V/K weights + xq
                xcs = {}
                xc0 = pxc.tile([128, 8, 512], bf16, tag="xc")
                nc.sync.dma_start(xc0[:], xT[:, :, 0:512])
                xcs[0] = xc0
                wvs = pw.tile([128, 8, KV], bf16, tag="wvs")
                nc.sync.dma_start(wvs[:], wv[:])
                wks = pw.tile([128, 8, KV], bf16, tag="wks")
                nc.sync.dma_start(wks[:], wk[:])
                xqs = px.tile([128, 8, OWN], bf16, tag="xqs")
                nc.sync.dma_start(xqs[:], xq[:])
                # long-lead tables and residuals after the critical batch
                nc.sync.dma_start(rmt[:], rmat[:])
                nc.sync.dma_start(coso[:], cosO[:])
                nc.sync.dma_start(sino[:], sinO[:])
                nc.sync.dma_start(cosf[:], cosF[:])
                nc.sync.dma_start(sinf[:], sinF[:])
                nc.sync.dma_start(xrs[:], xres[:])
                nc.sync.dma_start(xrs2[:], xres2[:])
                nc.sync.dma_start(gat[:], g_attn[:])
                nc.sync.dma_start(gml[:], g_mlp[:])

                def unit_v(ci):
                    # V token-major for the 4 token tiles of chunk ci
                    xc = xcs[ci]
                    for kt in range(4):
                        gkt = ci * 4 + kt
                        vps = pps.tile([128, KV], f32, tag="vps", bufs=1)
                        for k in range(8):
                            nc.tensor.matmul(vps[:], xc[:, k, kt * 128:(kt + 1) * 128],
                                             wvs[:, k, :], start=(k == 0), stop=(k == 7))
                        nc.scalar.activation(
                            v_all[:, :, gkt, 0:64],
                            vps[:].rearrange("p (g d) -> p g d", g=4), AF.Copy)

                def unit_proj(u):
                    kind = u[0]
                    if kind == "Q":
                        p = u[1]
                        wqs = pwq.tile([128, 8, 128], bf16, tag="wqs")
                        nc.sync.dma_start(wqs[:], wq[p])
                        ps = pps.tile([128, OWN], f32, tag="pps", bufs=3)
                        for k in range(8):
                            nc.tensor.matmul(ps[:], wqs[:, k, :], xqs[:, k, :],
                                             start=(k == 0), stop=(k == 7))
                    else:
                        ci, kp = u[1], u[2]
                        xc = xcs[ci]
                        ps = pps.tile([128, OWN], f32, tag="pps", bufs=3)
                        for k in range(8):
                            nc.tensor.matmul(ps[:], wks[:, k, kp * 128:(kp + 1) * 128],
                                             xc[:, k, :], start=(k == 0), stop=(k == 7))
                    sq = psb.tile([128, OWN], bf16, tag="sq")
                    nc.scalar.activation(sq[:], ps[:], AF.Square)
                    return (u, ps, sq)

                def unit_norm(st):
                    u, ps, sq = st
                    inv = psb.tile([2, 2, OWN], INV_DT, tag="inv")
                    for half in range(2):
                        hs = slice(half * 64, (half + 1) * 64)
                        ss = pss.tile([2, OWN], f32, tag="ss")
                        nc.tensor.matmul(ss[:], onesc[hs, 0:2], sq[hs, :],
                                         start=True, stop=True,
                                         tile_position=(half * 64, 0))
                        rsqrt_into(psb, inv[0:2, half, :], ss[0:2, :],
                                   epsq[0:2, :], "n")
                    rep = psb.tile([128, 2, OWN], INV_DT, tag="rep")
                    nc.gpsimd.partition_broadcast(rep[:], inv[0:1, :, :],
                                                  channels=128)
                    pbf = psb.tile([128, OWN], bf16, tag="pbf")
                    nc.scalar.activation(pbf[:], ps[:], AF.Copy)
                    if u[0] == "Q":
                        xn = pqn.tile([128, OWN], bf16, tag="qn", bufs=8)
                        for half in range(2):
                            hs = slice(half * 64, (half + 1) * 64)
                            nc.vector.scalar_tensor_tensor(
                                xn[hs, :], pbf[hs, :], gqt[hs, u[1], None],
                                rep[hs, half, :], ALU.mult, ALU.mult)
                    else:
                        xn = pqn.tile([128, OWN], bf16, tag="kn", bufs=4)
                        for half in range(2):
                            hs = slice(half * 64, (half + 1) * 64)
                            nc.vector.tensor_tensor(xn[hs, :], pbf[hs, :],
                                                    rep[hs, half, :], ALU.mult)
                    return (u, xn)

                def unit_rope(st):
                    u, xn = st
                    rot = prt.tile([128, OWN], f32, tag="rot")
                    nc.tensor.matmul(rot[:], rmt[:], xn[:], start=True, stop=True)
                    t1 = psb.tile([128, OWN], bf16, tag="t1")
                    t2 = psb.tile([128, OWN], bf16, tag="t2")
                    if u[0] == "Q":
                        nc.vector.tensor_tensor(t1[:], xn[:], coso[:], ALU.mult)
                        nc.vector.tensor_tensor(t2[:], rot[:], sino[:], ALU.mult)
                        nc.vector.tensor_tensor(qT[:, u[1], :], t1[:], t2[:],
                                                ALU.add)
                    else:
                        ci, kp = u[1], u[2]
                        sl = slice(ci * 512, (ci + 1) * 512)
                        nc.vector.tensor_tensor(t1[:], xn[:], cosf[:, sl], ALU.mult)
                        nc.vector.tensor_tensor(t2[:], rot[:], sinf[:, sl], ALU.mult)
                        nc.vector.tensor_tensor(kT[:, kp, sl], t1[:], t2[:],
                                                ALU.add)

                units = []
                qp = 0
                for ci in range(4):
                    units.append(("V", ci))
                    units.append(("K", ci, 0))
                    units.append(("K", ci, 1))
                    units.append(("Q", qp)); qp += 1
                    units.append(("Q", qp)); qp += 1

                normq, ropeq = [], []
                for u in units:
                    if u[0] == "V":
                        ci = u[1]
                        if ci + 1 < 4:
                            xcn = pxc.tile([128, 8, 512], bf16, tag="xc")
                            nc.sync.dma_start(xcn[:],
                                              xT[:, :, (ci + 1) * 512:(ci + 2) * 512])
                            xcs[ci + 1] = xcn
                        unit_v(ci)
                        continue
                    normq.append(unit_proj(u))
                    if len(normq) >= 2:
                        ropeq.append(unit_norm(normq.pop(0)))
                    if len(ropeq) >= 3:
                        unit_rope(ropeq.pop(0))
                while normq:
                    ropeq.append(unit_norm(normq.pop(0)))
                while ropeq:
                    unit_rope(ropeq.pop(0))

        rope_stack.close()

        # ------------- Phase D: attention ---------------------------------
        wo_stack = ExitStack()
        pe_w = wo_stack.enter_context(tc.tile_pool(name="pe_w", bufs=1))
        wosl = []
        for o in range(8):
            wos = pe_w.tile([128, 8, 128], bf16, tag=f"wos{o}")
            wosl.append(wos)
        if ph_on("d"):
            for o in range(8):
                nc.sync.dma_start(wosl[o][:], wo[o])
            with tc.tile_pool(name="pd_m", bufs=1) as pd_m, \
                 tc.tile_pool(name="pd_pt", bufs=6) as pd_pt, \
                 tc.tile_pool(name="pd_sb", bufs=3) as pd_sb, \
                 tc.tile_pool(name="pd_s", bufs=2, space="PSUM") as pd_s, \
                 tc.tile_pool(name="pd_y", bufs=2, space="PSUM") as pd_y:
                masks = pd_m.tile([128, 4, 4, 128], bf16, tag="masks")
                nc.sync.dma_start(masks[:], maskM[:])
                for t in range(4):
                    qsl = slice(t * 128, (t + 1) * 128)
                    n_chunks = t + 1
                    n_kvt = 4 * n_chunks
                    for half in range(2):
                        gA, gB = 2 * half, 2 * half + 1
                        yA = pd_y.tile([66, 4, 128], f32, tag="yA")
                        yB = pd_y.tile([66, 4, 128], f32, tag="yB")
                        qsA = qT[0:64, 4 * half:4 * half + 4, qsl]
                        qsB = qT[64:128, 4 * half:4 * half + 4, qsl]
                        av_pend = []

                        def emit_av(c, i2, ptA, ptB):
                            for isub in range(2):
                                kvt = 4 * c + 2 * i2 + isub
                                nc.tensor.matmul(yA[:], v_all[:, gA, kvt, :],
                                                 ptA[:, isub, :, :],
                                                 start=(kvt == 0),
                                                 stop=(kvt == n_kvt - 1))
                                nc.tensor.matmul(yB[:], v_all[:, gB, kvt, :],
                                                 ptB[:, isub, :, :],
                                                 start=(kvt == 0),
                                                 stop=(kvt == n_kvt - 1))

                        for c in range(n_chunks):
                            for i2 in range(2):
                                sA = pd_s.tile([128, 2, 4, 128], f32, tag="s")
                                sB = pd_s.tile([128, 2, 4, 128], f32, tag="s")
                                for isub in range(2):
                                    i = 2 * i2 + isub
                                    ks = slice((4 * c + i) * 128,
                                               (4 * c + i + 1) * 128)
                                    nc.tensor.matmul(sA[:, isub, :, :],
                                                     kT[0:64, half, ks], qsA,
                                                     start=True, stop=True,
                                                     tile_position=(0, 0))
                                    nc.tensor.matmul(sB[:, isub, :, :],
                                                     kT[64:128, half, ks], qsB,
                                                     start=True, stop=True,
                                                     tile_position=(64, 0))
                                ptA = pd_pt.tile([128, 2, 4, 128], bf16, tag="ptA")
                                ptB = pd_pt.tile([128, 2, 4, 128], bf16, tag="ptB")
                                nc.scalar.activation(ptA[:], sA[:], AF.Exp,
                                                     scale=0.125)
                                nc.scalar.activation(ptB[:], sB[:], AF.Exp,
                                                     scale=0.125)
                                if c == t:
                                    mbc = masks[:, t, 2 * i2:2 * i2 + 2, None, :] \
                                        .to_broadcast([128, 2, 4, 128])
                                    nc.vector.tensor_tensor(ptA[:], ptA[:], mbc,
                                                            ALU.mult)
                                    nc.vector.tensor_tensor(ptB[:], ptB[:], mbc,
                                                            ALU.mult)
                                av_pend.append((c, i2, ptA, ptB))
                                if len(av_pend) > 1:
                                    emit_av(*av_pend.pop(0))
                        while av_pend:
                            emit_av(*av_pend.pop(0))
                        for ab, y in ((0, yA), (1, yB)):
                            dn = pd_sb.tile([2, 4, 128], f32, tag="dn")
                            nc.vector.tensor_copy(dn[0:2, :, :], y[64:66, :, :])
                            invy = pd_sb.tile([2, 4, 128], f32, tag="invy")
                            nc.vector.reciprocal_approx_fast(
                                out=invy[:], in_=dn[:])
                            repy = pd_sb.tile([128, 4, 128], f32, tag="repy")
                            nc.gpsimd.partition_broadcast(
                                repy[:], invy[0:1, :, :], channels=128)
                            c20 = 4 * half + 2 * ab
                            y4 = y.rearrange("p (a b) q -> p a b q", b=2)
                            r4 = repy.rearrange("p (a b) q -> p a b q", b=2)
                            for ph2 in range(2):
                                nc.vector.tensor_tensor(
                                    y_all[ph2 * 64:ph2 * 64 + 64,
                                          c20:c20 + 2, qsl],
                                    y4[0:64, :, ph2, :],
                                    r4[0:64, :, ph2, :], ALU.mult)

        # ------------- Phase E: Wo + post-norm + residual -----------------
        if ph_on("e"):
            xpr = big.tile([128, 8, OWN], bf16, tag="qT_xpr")
            xpb = big.tile([128, 8, OWN], f32, tag="xpb")
            with tc.tile_pool(name="pe_sb", bufs=2) as pe_sb, \
                 tc.tile_pool(name="pe_ao", bufs=1) as pe_ao, \
                 tc.tile_pool(name="pe_ps", bufs=3, space="PSUM") as pe_ps, \
                 tc.tile_pool(name="pe_ps1", bufs=1, space="PSUM") as pe_ps1:
                ao = pe_ao.tile([128, 8, OWN], f32, tag="ao")
                ssa = pe_ps1.tile([2, OWN], f32, tag="ssa")
                a2l = []
                for o in range(8):
                    wos = wosl[o]
                    aps = pe_ps.tile([128, OWN], f32, tag="aps")
                    for k in range(8):
                        nc.tensor.matmul(aps[:], wos[:, k, :], y_all[:, k, :],
                                         start=(k == 0), stop=(k == 7))
                    nc.scalar.activation(ao[:, o, :], aps[:], AF.Copy)
                    a2 = pe_sb.tile([128, OWN], bf16, tag="a2", bufs=3)
                    nc.scalar.activation(a2[:], aps[:], AF.Square)
                    a2l.append(a2)
                    if o >= 1:
                        nc.tensor.matmul(ssa[:], onesc[:, 0:2], a2l[o - 1][:],
                                         start=(o == 1), stop=False)
                nc.tensor.matmul(ssa[:], onesc[:, 0:2], a2l[7][:],
                                 start=False, stop=True)
                inva = pe_sb.tile([2, OWN], INV_DT, tag="inva")
                if USE_ABSRSQRT:
                    nc.scalar.activation(inva[:], ssa[0:2, :],
                                         AF.Abs_reciprocal_sqrt,
                                         scale=1.0 / D, bias=eps6[0:2, :])
                else:
                    rmsa = pe_sb.tile([2, OWN], f32, tag="rmsa")
                    nc.scalar.activation(rmsa[:], ssa[0:2, :], AF.Sqrt,
                                         scale=1.0 / D, bias=eps6[0:2, :])
                    nc.vector.reciprocal_approx_fast(out=inva[:], in_=rmsa[:])
                repa = pe_sb.tile([128, OWN], INV_DT, tag="repa")
                nc.gpsimd.partition_broadcast(repa[:], inva[0:1, :], channels=128)
                tmps = []
                for o in range(8):
                    tmp = pe_sb.tile([128, OWN], f32, tag="tmpe", bufs=8)
                    nc.vector.scalar_tensor_tensor(
                        tmp[:], ao[:, o, :], gat[:, o, None], repa[:],
                        ALU.mult, ALU.mult)
                    nc.vector.tensor_tensor(xpr[:, o, :], tmp[:], xrs[:, o, :],
                                            ALU.add)
                    tmps.append(tmp)
                for o in range(8):
                    nc.gpsimd.tensor_tensor(xpb[:, o, :], tmps[o][:],
                                            xrs2[:, o, :], ALU.add)
            wo_stack.close()

        # ------------- Phase F: MLP ---------------------------------------
        if ph_on("f"):
            mout = big.tile([128, 8, OWN], f32, tag="xrs2_mout")
            with tc.tile_pool(name="pf_h2", bufs=1) as pf_h2, \
                 tc.tile_pool(name="pf_sb", bufs=3) as pf_sb, \
                 tc.tile_pool(name="pf_wf", bufs=3) as pf_wf, \
                 tc.tile_pool(name="pf_wp", bufs=3) as pf_wp, \
                 tc.tile_pool(name="pf_ps", bufs=2, space="PSUM") as pf_ps, \
                 tc.tile_pool(name="pf_mo", bufs=1, space="PSUM") as pf_mo, \
                 tc.tile_pool(name="pf_ss", bufs=1, space="PSUM") as pf_ss:
                h2 = pf_h2.tile([128, 32, OWN], bf16, tag="h2")
                ssm = pf_ss.tile([2, OWN], f32, tag="ssm")

                def emit_fc(hc):
                    wfs = pf_wf.tile([128, 8, 128], bf16, tag="wfs")
                    nc.sync.dma_start(wfs[:], wfc[hc])
                    hps = pf_ps.tile([128, OWN], f32, tag="hps")
                    for k in range(8):
                        nc.tensor.matmul(hps[:], wfs[:, k, :], xpr[:, k, :],
                                         start=(k == 0), stop=(k == 7))
                    hr = pf_sb.tile([128, OWN], bf16, tag="hr")
                    nc.scalar.activation(hr[:], hps[:], AF.Relu)
                    nc.vector.tensor_tensor(h2[:, hc, :], hr[:], hr[:], ALU.mult)

                mo_all = []
                for ohalf in range(2):
                    mo_ps = [pf_mo.tile([128, OWN], f32, name=f"mo{oi}",
                                        tag=f"mo{oi}") for oi in range(4)]
                    mo_all.append(mo_ps)

                wp1 = [pf_wp.tile([128, 4, 128], bf16, name=f"wp1_{hc}",
                                  tag=f"wp1_{hc}", bufs=1) for hc in range(32)]
                for hc in range(34):
                    if hc < 32:
                        emit_fc(hc)
                    if hc == 2:
                        # preload all second-half proj weights during fc phase
                        for hj in range(32):
                            nc.sync.dma_start(wp1[hj][:], wproj[hj, 1])
                    if hc >= 2:
                        hp = hc - 2
                        wps = pf_wp.tile([128, 4, 128], bf16, tag="wps")
                        nc.sync.dma_start(wps[:], wproj[hp, 0])
                        for oi in range(4):
                            nc.tensor.matmul(mo_all[0][oi][:], wps[:, oi, :],
                                             h2[:, hp, :],
                                             start=(hp == 0), stop=(hp == 31))
                # ohalf0 copies/squares run on scalar during the proj1 matmuls
                m2l = []
                for oi in range(4):
                    nc.scalar.activation(mout[:, oi, :], mo_all[0][oi][:],
                                         AF.Copy)
                    m2 = pf_sb.tile([128, OWN], bf16, tag="m2", bufs=8)
                    nc.scalar.activation(m2[:], mo_all[0][oi][:], AF.Square)
                    m2l.append(m2)
                # proj1 per output group so each group's norm work starts early
                for oi in range(4):
                    for hc in range(32):
                        nc.tensor.matmul(mo_all[1][oi][:], wp1[hc][:, oi, :],
                                         h2[:, hc, :],
                                         start=(hc == 0), stop=(hc == 31))
                    if oi < 4:
                        nc.tensor.matmul(ssm[:], onesc[:, 0:2], m2l[oi][:],
                                         start=(oi == 0), stop=False)
                    m2 = pf_sb.tile([128, OWN], bf16, tag="m2", bufs=8)
                    nc.scalar.activation(m2[:], mo_all[1][oi][:], AF.Square)
                    m2l.append(m2)
                for o in range(4, 8):
                    nc.tensor.matmul(ssm[:], onesc[:, 0:2], m2l[o][:],
                                     start=False, stop=(o == 7))
                invm = pf_sb.tile([2, OWN], INV_DT, tag="invm")
                if USE_ABSRSQRT:
                    nc.scalar.activation(invm[:], ssm[0:2, :],
                                         AF.Abs_reciprocal_sqrt,
                                         scale=1.0 / D, bias=eps6[0:2, :])
                else:
                    rmsm = pf_sb.tile([2, OWN], f32, tag="rmsm")
                    nc.scalar.activation(rmsm[:], ssm[0:2, :], AF.Sqrt,
                                         scale=1.0 / D, bias=eps6[0:2, :])
                    nc.vector.reciprocal_approx_fast(out=invm[:], in_=rmsm[:])
                repm = pf_sb.tile([128, OWN], INV_DT, tag="repm")
                nc.gpsimd.partition_broadcast(repm[:], invm[0:1, :], channels=128)
                for o in range(8):
                    msrc = mout[:, o, :] if o < 4 else mo_all[1][o - 4][:]
                    tmp = pf_sb.tile([128, OWN], f32, tag="tmpf")
                    nc.vector.scalar_tensor_tensor(
                        tmp[:], msrc, gml[:, o, None], repm[:],
                        ALU.mult, ALU.mult)
                    outv = pf_sb.tile([128, OWN], f32, tag="outv", bufs=4)
                    eng = nc.vector if o % 2 == 0 else nc.gpsimd
                    eng.tensor_tensor(outv[:], tmp[:], xpb[:, o, :], ALU.add)
                    nc.sync.dma_start(out_t[:, o, :], outv[:])

    nc.finalize()
    return nc


def _feat_major(a):
    """[F, T] -> device layout [128, F//128, T]."""
    F, T = a.shape
    return np.ascontiguousarray(a.reshape(F // 128, 128, T).transpose(1, 0, 2))


def _vec_dev(v):
    return np.ascontiguousarray(v.reshape(-1, 128).T)


def _bf(a):
    return np.ascontiguousarray(np.asarray(a, np.float32)).astype(ml_dtypes.bfloat16)


_CACHE = {}
_RUN_KW = {}



def kernel(x, attn_norm_w, mlp_norm_w, attn_post_norm_w, mlp_post_norm_w,
           attn_scale, mlp_scale, attn_mod_gain, attn_mod_bias,
           mlp_mod_gain, mlp_mod_bias, Wq, Wk, Wv, Wo, q_gain, fc_w, proj_w):
    x = np.asarray(x, np.float32)
    q_gain = np.asarray(q_gain, np.float32)

    if "nc" not in _CACHE:
        _CACHE["nc"] = build()
    nc = _CACHE["nc"]

    anw = np.asarray(attn_norm_w, np.float32)
    mnw = np.asarray(mlp_norm_w, np.float32)
    fc_eff = np.asarray(fc_w, np.float32) * mnw[None, :]

    # host-side input rmsnorm: xn = x * anw / rms1(x)
    ms1 = np.mean(np.square(x), axis=-1, keepdims=True)
    xn = (x * (1.0 / np.sqrt(ms1 + EPS_BLOCK))) * anw[None, None, :]

    # Wq columns permuted so p-tile p holds heads PAIRS[p] stacked (64+64)
    perm = np.zeros(D, np.int64)
    for p, (a, b) in enumerate(PAIRS):
        perm[p * 128:p * 128 + 64] = np.arange(a * 64, a * 64 + 64)
        perm[p * 128 + 64:(p + 1) * 128] = np.arange(b * 64, b * 64 + 64)
    WqTp = np.asarray(Wq, np.float32).T[:, perm]
    wq_dev = np.stack([_feat_major(WqTp[:, p * 128:(p + 1) * 128]) for p in range(8)])
    wk_dev = _feat_major(np.asarray(Wk, np.float32).T)
    wv_dev = _feat_major(np.asarray(Wv, np.float32).T)

    # Wo rows permuted to match y_all layout: chunk c2 = 4*half + 2*ab + i//2,
    # partition ph2*64+f  ->  original feature 64*(4*(2*half+ab) + i) + f,
    # with i = 2*(c2 % 2) + ph2.
    perm2 = np.zeros(D, np.int64)
    for c2 in range(8):
        halfg = c2 // 2          # kv-head index (2*half + ab)
        for ph2 in range(2):
            i = 2 * (c2 % 2) + ph2
            h_orig = 4 * halfg + i
            rows = np.arange(64)
            perm2[c2 * 128 + ph2 * 64 + rows] = 64 * h_orig + rows
    WoT = np.asarray(Wo, np.float32).T[perm2, :]
    wo_dev = np.stack([_feat_major(WoT[:, o * 128:(o + 1) * 128]) for o in range(8)])

    fcT = fc_eff.T
    wfc_dev = np.stack([_feat_major(fcT[:, h * 128:(h + 1) * 128]) for h in range(32)])
    projT = np.asarray(proj_w, np.float32).T                  # [4096, 1024]
    wproj_dev = np.ascontiguousarray(
        projT.reshape(32, 128, 2, 4, 128).transpose(0, 2, 1, 3, 4))

    # rope tables; sin sign-folded: x1-groups (even 32-blocks) get -sin
    inv_freq = 1.0 / (ROPE_BASE ** (np.arange(0, HD, 2, dtype=np.float32) / HD))
    tpos = np.arange(S, dtype=np.float32)
    freqs = np.outer(tpos, inv_freq).astype(np.float32)
    cosT = np.ascontiguousarray(np.tile(np.cos(freqs).T, (4, 1)))   # [128, S]
    sinN = np.ascontiguousarray(np.tile(np.sin(freqs).T, (4, 1)))
    # rope rotate-half matrix with signs: rot = R @ x (per 64-feature head)
    R = np.zeros((128, 128), np.float32)
    for p in range(128):
        if p % 64 < 32:
            R[p, p + 32] = 1.0
        else:
            R[p, p - 32] = -1.0
    rmat_h = np.ascontiguousarray(R.T)

    gat_v = (np.asarray(attn_post_norm_w, np.float32)
             * np.asarray(attn_mod_gain, np.float32)
             * np.asarray(attn_scale, np.float32))
    bat_v = np.asarray(attn_mod_bias, np.float32) * np.asarray(attn_scale, np.float32)
    gml_v = (np.asarray(mlp_post_norm_w, np.float32)
             * np.asarray(mlp_mod_gain, np.float32)
             * np.asarray(mlp_scale, np.float32))
    bml_v = np.asarray(mlp_mod_bias, np.float32) * np.asarray(mlp_scale, np.float32)

    gq_h = np.zeros((128, 8), np.float32)
    for p, (a, b) in enumerate(PAIRS):
        gq_h[0:64, p] = q_gain[a]
        gq_h[64:128, p] = q_gain[b]

    shared = {
        "wq": _bf(wq_dev), "wk": _bf(wk_dev), "wv": _bf(wv_dev),
        "wo": _bf(wo_dev), "wfc": _bf(wfc_dev), "wproj": _bf(wproj_dev),
        "cosF": _bf(cosT), "sinF": _bf(sinN), "rmat": _bf(rmat_h),
        "ones_c": _bf(np.ones((128, 2), np.float32)),
        "gq": gq_h,
        "g_attn": _vec_dev(gat_v), "g_mlp": _vec_dev(gml_v),
    }

    in_maps = []
    owners = []
    for c in range(8):
        b, j = c // 4, c % 4
        rows = np.concatenate(
            [np.arange((j + 4 * t) * 128, (j + 4 * t + 1) * 128) for t in range(4)])
        owners.append((b, rows))
        xnb = xn[b].T
        x_own_raw = x[b].T[:, rows]
        mask = np.zeros((4, 4, 128, 128), np.float32)
        for t in range(4):
            m = j + 4 * t
            q_idx = m * 128 + np.arange(128)
            for ktl in range(4):
                kv_idx = 512 * t + 128 * ktl + np.arange(128)
                mask[t, ktl] = (kv_idx[:, None] <= q_idx[None, :])
        m_in = {
            "xT": _bf(_feat_major(xnb)),
            "xq": _bf(_feat_major(xnb[:, rows])),
            "xres": _feat_major(x_own_raw + bat_v[:, None]),
            "xres2": _feat_major(x_own_raw + (bat_v + bml_v)[:, None]),
            "cosO": _bf(cosT[:, rows]),
            "sinO": _bf(sinN[:, rows]),
            "maskM": _bf(np.ascontiguousarray(mask.transpose(2, 0, 1, 3))),
        }
        m_in.update(shared)
        in_maps.append(m_in)

    res = run_bass_kernel_spmd(nc, in_maps, core_ids=list(range(8)),
                               **_RUN_KW)
    _CACHE["last_result"] = res

    out = np.empty((B, S, D), np.float32)
    for c in range(8):
        b, rows = owners[c]
        o = res.results[c]["out"]
        out[b, rows, :] = o.transpose(2, 1, 0).reshape(OWN, D)
    return out
